# revision 1
# baseline (speedup 1.0000x reference)
"""AttnBlock3d on 8 TRN2 NeuronCores.

Sharding: 8 cores = 4 batches x 2 query-halves. Each core receives its
batch's full x (rotated so its query half is always voxels [0:2048] --
GroupNorm and the attention key-reduction are voxel-permutation
invariant, so all cores run an identical graph), computes GN stats +
QKV + full attention for its 2048 queries and writes a [2,128,2048]
channel-tiled chunk.

Math restructuring vs the reference (exact up to fp rounding):
- GroupNorm apply is folded into the projections: hn = a*x + b with
  per-channel a = gamma*rstd, b = beta - mean*a.  Projections run on
  RAW x with column-scaled weights W* = W diag(a) (computed on-chip by
  a per-partition scale of the transposed weight tiles).  The b-terms
  reduce to per-query score shifts (softmax-invariant, dropped) plus a
  per-key term captured by keeping the full q bias cq = Wq b + bq
  (computed on-chip with tiny N=1 matmuls).  k needs no bias at all.
- The output projection is fused into V: values vv = (Wo Wv) diag(a) x
  (Wo@Wv precomputed on host), so the PV matmul directly produces the
  o-projected output; the remaining constant (WoWv b + Wo bv + bo) is
  added at the end (softmax weights sum to 1).
- Scores are computed transposed (S^T = k'^T q, fp32r) so no transposes
  are needed; exp applies a constant shift (scores stay in [-97,97]).
  The softmax denominator accumulates P^T tiles on Vector/GpSimd (one
  chunk each), summed across partitions by a ones-matmul; normalization
  is applied at the very end (linearity).
- The attention loop processes TWO 512-query chunks per key-tile pass so
  each stationary tile serves 2 matmuls (halves LDWEIGHTS exposure).
- The DMA-load head warms the PE (HAM) with junk matmuls paced by x
  chunk arrivals, and preloads the sqrt activation table with a dummy
  so neither table load lands on the critical path.
"""

import sys

for _p in ("/opt/trn_rl_repo",):
    if _p not in sys.path:
        sys.path.append(_p)

import numpy as np

B, C, DD, HH, WW = 4, 256, 16, 16, 16
N = DD * HH * WW          # 4096 voxels
NQ = N // 2               # queries per core
GROUPS = 32
CPG = C // GROUPS         # channels per group
EPS = 1e-6
SHIFT = 60.0              # softmax constant shift
NCORES = 8
IC = 512                  # query chunk
NIC = NQ // IC            # 4 chunks
NJT = N // 128            # 32 key tiles
XC = 1024                 # x-load / GN chunk
NXC = N // XC

# packed-constant column offsets (constR: f32r, constF: f32)
_RQ, _RK, _RO = 0, 512, 1024
_RCOLS = 1536
_FGS, _FGT, _FVEC, _FKC = 0, 64, 320, 328
_FCOLS = 330

_cache = {}


def _build():
    import concourse.bass as bass
    from concourse import bacc, mybir, tile

    f32 = mybir.dt.float32
    f32r = mybir.dt.float32r
    bf16 = mybir.dt.bfloat16
    AF = mybir.ActivationFunctionType
    OP = mybir.AluOpType
    AX = mybir.AxisListType

    nc = bacc.Bacc("TRN2", target_bir_lowering=False, debug=False,
                   num_devices=NCORES)

    x_e = nc.dram_tensor("x", [2, 128, N], f32r, kind="ExternalInput").ap()
    cR_e = nc.dram_tensor("constR", [128, _RCOLS], f32r,
                          kind="ExternalInput").ap()
    cF_e = nc.dram_tensor("constF", [128, _FCOLS], f32,
                          kind="ExternalInput").ap()
    out_e = nc.dram_tensor("out", [2, 128, NQ], f32, kind="ExternalOutput").ap()

    with tile.TileContext(nc) as tc:
        with tc.tile_pool(name="big", bufs=1) as big, \
             tc.tile_pool(name="w", bufs=1) as wp, \
             tc.tile_pool(name="sm", bufs=2) as sm, \
             tc.tile_pool(name="pt", bufs=8) as ptp, \
             tc.tile_pool(name="res", bufs=2) as resp, \
             tc.tile_pool(name="psum", bufs=1, space="PSUM") as ps:

            # ---- on-chip constants (no DMA dependency) ----
            ones128 = wp.tile([128, 128], f32r, tag="ones", name="ones128")
            ones_h = wp.tile([128, 128], bf16, tag="onesh", name="ones_h")
            ones_f = wp.tile([128, 128], f32, tag="onesf", name="ones_f")
            nc.vector.memset(ones_f[:], 1.0)
            nc.vector.tensor_copy(ones128[:], ones_f[:])
            nc.vector.tensor_copy(ones_h[:], ones_f[:])
            dzi = sm.tile([GROUPS, 1], f32, tag="dzi", name="dzi")
            dzo = sm.tile([GROUPS, 1], f32, tag="dzo", name="dzo")
            nc.vector.memset(dzi[:], 0.25)
            # preload the sqrt activation table off the critical path
            nc.scalar.activation(dzo[:], dzi[:], AF.Sqrt)

            # ---- x load (chunked, issued first; separate tiles so GN
            # partial reductions start as soon as each chunk lands) ----
            xc = [[big.tile([128, XC], f32r, tag=f"x{t}_{cx}", name=f"x{t}_{cx}")
                   for cx in range(NXC)] for t in range(2)]
            for cx in range(NXC):
                for t in range(2):
                    sl = slice(cx * XC, (cx + 1) * XC)
                    nc.sync.dma_start(xc[t][cx][:], x_e[t, :, sl])

            # ---- packed constants ----
            cR = wp.tile([128, _RCOLS], f32r, tag="cR", name="cR")
            cF = wp.tile([128, _FCOLS], f32, tag="cF", name="cF")
            nc.sync.dma_start(cR[:], cR_e[:])
            nc.sync.dma_start(cF[:], cF_e[:])
            # raw transposed weights [c_in, c_out]; blocks (2t+m)
            wqT = [[cR[:, _RQ + 128 * (2 * t + m):_RQ + 128 * (2 * t + m) + 128]
                    for m in range(2)] for t in range(2)]
            wovT = [cR[:, _RO + 256 * t:_RO + 256 * t + 256] for t in range(2)]
            gsel = [cF[:, _FGS + 32 * t:_FGS + 32 * t + 32] for t in range(2)]
            gselT = [cF[0:GROUPS, _FGT + 128 * t:_FGT + 128 * t + 128]
                     for t in range(2)]
            gamma = [cF[:, _FVEC + 0 + t:_FVEC + 1 + t] for t in range(2)]
            beta = [cF[:, _FVEC + 2 + t:_FVEC + 3 + t] for t in range(2)]
            bq = [cF[:, _FVEC + 4 + t:_FVEC + 5 + t] for t in range(2)]
            wobvbo = [cF[:, _FVEC + 6 + t:_FVEC + 7 + t] for t in range(2)]
            kconst = cF[:, _FKC:_FKC + 2]

            # ---- GroupNorm stats (chunked, overlapping the x load), plus
            # bf16 x copies (vv-proj stationaries) and PE warm-up matmuls
            # paced by chunk arrivals (keeps HAM at 8/8) ----
            xb = [[big.tile([128, XC], bf16, tag=f"xb{t}_{cx}",
                            name=f"xb{t}_{cx}") for cx in range(NXC)]
                  for t in range(2)]
            sum4 = [sm.tile([128, NXC], f32, tag=f"sum4{t}", name=f"sum4{t}")
                    for t in range(2)]
            sq4 = [sm.tile([128, NXC], f32, tag=f"sq4{t}", name=f"sq4{t}")
                   for t in range(2)]
            stats = [sm.tile([128, 2], f32, tag=f"st{t}", name=f"st{t}")
                     for t in range(2)]
            for cx in range(NXC):
                for t in range(2):
                    nc.scalar.activation(xb[t][cx][:], xc[t][cx][:],
                                         AF.Identity,
                                         accum_out=sum4[t][:, cx:cx + 1])
                    sqs = sm.tile([128, XC], bf16, tag="sqscr", name="sqscr")
                    nc.scalar.activation(sqs[:], xc[t][cx][:], AF.Square,
                                         accum_out=sq4[t][:, cx:cx + 1])
                    w_ps = ps.tile([128, IC], f32, tag="s", name="warm", bufs=4)
                    nc.tensor.matmul(w_ps[:], ones128, xc[t][cx][:, 0:IC],
                                     start=True, stop=True)
            # dense warm burst on the last chunk to flip HAM before the
            # real stream starts
            for r in range(10):
                t, co = r % 2, (r % 5) * 100
                w_ps = ps.tile([128, IC], f32, tag="s", name="warm", bufs=4)
                nc.tensor.matmul(w_ps[:], ones128, xc[t][NXC - 1][:, co:co + IC],
                                 start=True, stop=True)

            def warmmm(n):
                for r in range(n):
                    t, co = r % 2, (r % 4) * 128
                    w_ps = ps.tile([128, IC], f32, tag="s", name="warm",
                                   bufs=4)
                    nc.tensor.matmul(w_ps[:], ones128,
                                     xc[t][NXC - 2][:, co:co + IC],
                                     start=True, stop=True)

            # ---- stats combine -> per-channel a, b ----
            for t in range(2):
                nc.vector.reduce_sum(stats[t][:, 0:1], sum4[t][:], axis=AX.X)
                nc.vector.reduce_sum(stats[t][:, 1:2], sq4[t][:], axis=AX.X)
            g_ps = ps.tile([GROUPS, 2], f32, tag="s", name="g_ps", bufs=4)
            for t in range(2):
                nc.tensor.matmul(g_ps[:], gsel[t], stats[t][:],
                                 start=(t == 0), stop=(t == 1))
            for t in range(2):
                f_ps = ps.tile([128, 2], f32, tag="s", name="fill", bufs=4)
                nc.tensor.matmul(f_ps[:], ones_f, stats[t][:],
                                 start=True, stop=True)
                warmmm(2)
            gstats = sm.tile([GROUPS, 2], f32, tag="gstats", name="gstats")
            var = sm.tile([GROUPS, 1], f32, tag="gvar", name="gvar")
            stdt = sm.tile([GROUPS, 1], f32, tag="gstd", name="gstd")
            inv = 1.0 / (CPG * N)
            nc.vector.tensor_scalar_mul(gstats[:, 0:2], g_ps[:, 0:2], inv)
            # negvar = mean*mean - ex2 ; std = sqrt(-negvar + eps)
            nc.vector.scalar_tensor_tensor(var[:], gstats[:, 0:1],
                                           gstats[:, 0:1], gstats[:, 1:2],
                                           op0=OP.mult, op1=OP.subtract)
            nc.scalar.activation(stdt[:], var[:], AF.Sqrt,
                                 bias=kconst[0:GROUPS, 1:2], scale=-1.0)
            nc.vector.reciprocal_approx_fast(gstats[:, 1:2], stdt[:])
            ab = [sm.tile([128, 2], f32, tag=f"ab{t}", name=f"ab{t}")
                  for t in range(2)]
            bcol = [sm.tile([128, 2], f32r, tag=f"bcol{t}", name=f"bcol{t}")
                    for t in range(2)]
            for t in range(2):
                bc_ps = ps.tile([128, 2], f32, tag="s", name="bc_ps", bufs=4)
                nc.tensor.matmul(bc_ps[:], gselT[t], gstats[:],
                                 start=True, stop=True)
                # a = rstd*gamma ; b = beta - mean*a
                nc.vector.tensor_mul(ab[t][:, 0:1], bc_ps[:, 1:2], gamma[t])
                nc.vector.tensor_mul(ab[t][:, 1:2], bc_ps[:, 0:1], ab[t][:, 0:1])
                nc.vector.tensor_sub(ab[t][:, 1:2], beta[t], ab[t][:, 1:2])
                nc.vector.tensor_copy(bcol[t][:, 0:1], ab[t][:, 1:2])
                nc.vector.tensor_copy(bcol[t][:, 1:2], ab[t][:, 1:2])

            f_ps = ps.tile([128, 2], f32, tag="s", name="fill", bufs=4)
            nc.tensor.matmul(f_ps[:], ones_f[0:GROUPS, :], gstats[:],
                             start=True, stop=True)
            warmmm(3)
            # ---- fold a into the weights: W* = W diag(a) (row-scale of the
            # transposed tiles); vv weights go to bf16 for FWL ----
            wks = [wp.tile([128, 256], f32r, tag=f"wks{t}", name=f"wks{t}")
                   for t in range(2)]
            wqs = [wp.tile([128, 256], f32r, tag=f"wqs{t}", name=f"wqs{t}")
                   for t in range(2)]
            wovs = [wp.tile([128, 256], bf16, tag=f"wovs{t}", name=f"wovs{t}")
                    for t in range(2)]
            for t in range(2):
                nc.vector.tensor_scalar_mul(wks[t][:],
                                            cR[:, _RK + 256 * t:_RK + 256 * t + 256],
                                            ab[t][:, 0:1])
            for t in range(2):
                nc.vector.tensor_scalar_mul(wqs[t][:],
                                            cR[:, _RQ + 256 * t:_RQ + 256 * t + 256],
                                            ab[t][:, 0:1])
            for t in range(2):
                nc.vector.tensor_scalar_mul(wovs[t][:], wovT[t], ab[t][:, 0:1])

            # ---- on-chip bias columns: cq = Wq b + bq (per attention-channel
            # tile), bfin = WoWv b + (Wo bv + bo) ----
            f_ps = ps.tile([128, 256], f32, tag="s", name="fill", bufs=4)
            nc.tensor.matmul(f_ps[:], ones128, wks[0][:], start=True, stop=True)
            warmmm(3)
            cqc = [sm.tile([128, 1], f32, tag=f"cq{m}", name=f"cq{m}")
                   for m in range(2)]
            bfin = [sm.tile([128, 1], f32, tag=f"bf{m}", name=f"bf{m}")
                    for m in range(2)]
            for m in range(2):
                c_ps = ps.tile([128, 2], f32, tag="s", name="c_ps", bufs=4)
                for t in range(2):
                    nc.tensor.matmul(c_ps[:], wqT[t][m], bcol[t][:],
                                     start=(t == 0), stop=(t == 1))
                nc.vector.tensor_scalar_add(cqc[m][:], c_ps[:, 0:1], bq[m])
            for m in range(2):
                c_ps = ps.tile([128, 2], f32, tag="s", name="c_ps", bufs=4)
                for t in range(2):
                    nc.tensor.matmul(c_ps[:], wovT[t][:, m * 128:m * 128 + 128],
                                     bcol[t][:], start=(t == 0), stop=(t == 1))
                nc.vector.tensor_scalar_add(bfin[m][:], c_ps[:, 0:1], wobvbo[m])

            # ---- projections on raw x ----
            qt = [[big.tile([128, IC], f32r, tag=f"q{e}_{f}", name=f"q{e}_{f}")
                   for f in range(NIC)] for e in range(2)]
            kt = [[big.tile([128, 512], f32r, tag=f"k{e}_{f}", name=f"k{e}_{f}")
                   for f in range(N // 512)] for e in range(2)]
            # vv pairs: [vv(2u) | vv(2u+1)], each [128 keys, 256 ch] bf16
            vvp = [big.tile([128, 512], bf16, tag=f"vv{u}", name=f"vv{u}")
                   for u in range(NJT // 2)]

            def qproj(f):
                for e in range(2):
                    q_ps = ps.tile([128, IC], f32, tag="s", name="q_ps",
                                   bufs=4)
                    for t in range(2):
                        nc.tensor.matmul(
                            q_ps[:], wqs[t][:, e * 128:e * 128 + 128],
                            xc[t][f // 2][:, (f % 2) * 512:(f % 2 + 1) * 512],
                            start=(t == 0), stop=(t == 1))
                    nc.vector.tensor_scalar_add(qt[e][f][:], q_ps[:], cqc[e])

            def kproj(f):
                for e in range(2):
                    k_ps = ps.tile([128, 512], f32, tag="s", name="k_ps",
                                   bufs=4)
                    for t in range(2):
                        nc.tensor.matmul(
                            k_ps[:], wks[t][:, e * 128:e * 128 + 128],
                            xc[t][f // 2][:, (f % 2) * 512:(f % 2 + 1) * 512],
                            start=(t == 0), stop=(t == 1))
                    nc.vector.tensor_copy(kt[e][f][:], k_ps[:])

            def vvproj(u):
                v_ps = ps.tile([128, 512], f32, tag="s", name="v_ps", bufs=4)
                for jj in range(2):
                    jt = 2 * u + jj
                    for t in range(2):
                        nc.tensor.matmul(
                            v_ps[:, jj * 256:jj * 256 + 256],
                            xb[t][jt // 8][:, (jt % 8) * 128:(jt % 8 + 1) * 128],
                            wovs[t][:], start=(t == 0), stop=(t == 1))
                nc.vector.tensor_copy(vvp[u][:], v_ps[:])

            kproj(0)
            kproj(1)
            qproj(0)
            qproj(1)
            vvproj(0)

            # ---- attention: 2 supers x 2 chunks x 32 key tiles ----
            lacc = [resp.tile([128, IC], f32r, tag=f"lacc{c}", name=f"lacc{c}",
                              bufs=1) for c in range(NIC)]
            osb = [[None, None] for c in range(NIC)]
            rx = [[None, None] for c in range(NIC)]
            plast = {}

            def make_rx(c):
                for m in range(2):
                    r = resp.tile([128, IC], f32, tag=f"rx{c}_{m}",
                                  name=f"rx{c}_{m}", bufs=1)
                    nc.vector.tensor_scalar_add(
                        r[:], xc[m][c // 2][:, (c % 2) * IC:(c % 2 + 1) * IC],
                        bfin[m])
                    rx[c][m] = r

            def finalize(sc, from_psum=None):
                for ci in range(2):
                    c = 2 * sc + ci
                    lbc_ps = ps.tile([128, IC], f32, tag="s", name="lbc_ps",
                                     bufs=4)
                    nc.tensor.matmul(lbc_ps[:], ones128, lacc[c][:],
                                     start=True, stop=False)
                    p30, p31 = plast[c]
                    nc.tensor.matmul(lbc_ps[:], ones_h, p30[:],
                                     start=False, stop=False)
                    nc.tensor.matmul(lbc_ps[:], ones_h, p31[:],
                                     start=False, stop=True)
                    rb = resp.tile([128, IC], f32, tag="rb", name="rb")
                    nc.vector.reciprocal_approx_fast(rb[:], lbc_ps[:])
                    for m in range(2):
                        src = osb[c][m] if from_psum is None \
                            else from_psum[ci][m]
                        scaled = resp.tile([128, IC], f32, tag="scaled",
                                           name="scaled")
                        nc.vector.tensor_mul(scaled[:], src[:], rb[:])
                        res = resp.tile([128, IC], f32, tag="res", name="res")
                        # deferred finalize keeps the add on GpSimd (hidden);
                        # the end-of-kernel finalize runs all-Vector: DVE adds
                        # are 2x faster and skip the cross-engine sync hop
                        eng = nc.gpsimd if from_psum is None else nc.vector
                        eng.tensor_add(res[:], scaled[:], rx[c][m][:])
                        nc.sync.dma_start(out_e[m, :, c * IC:(c + 1) * IC],
                                          res[:])

            for sc in range(2):
                ca, cb = 2 * sc, 2 * sc + 1
                pv_ps = [[ps.tile([128, IC], f32, tag=f"pv{ci}_{m}",
                                  name=f"pv{ci}_{m}", bufs=1)
                          for m in range(2)] for ci in range(2)]

                def scores_block(jt):
                    s_a = ps.tile([128, IC], f32, tag="s", name="s_a", bufs=4)
                    s_b = ps.tile([128, IC], f32, tag="s", name="s_b", bufs=4)
                    for e in range(2):
                        ktile = kt[e][jt // 4][:, (jt % 4) * 128:(jt % 4 + 1) * 128]
                        nc.tensor.matmul(s_a[:], ktile, qt[e][ca][:],
                                         start=(e == 0), stop=(e == 1))
                        nc.tensor.matmul(s_b[:], ktile, qt[e][cb][:],
                                         start=(e == 0), stop=(e == 1))
                    ptag = "pt" if jt < NJT - 2 else "pfin"
                    pbufs = {} if jt < NJT - 2 else {"bufs": 4}
                    p_a = ptp.tile([128, IC], bf16, tag=ptag, name=ptag,
                                   **pbufs)
                    nc.scalar.activation(p_a[:], s_a[:], AF.Exp,
                                         bias=kconst[:, 0:1])
                    p_b = ptp.tile([128, IC], bf16, tag=ptag, name=ptag,
                                   **pbufs)
                    nc.scalar.activation(p_b[:], s_b[:], AF.Exp,
                                         bias=kconst[:, 0:1])
                    if jt == NJT - 2:
                        plast[ca] = [p_a, None]
                        plast[cb] = [p_b, None]
                    elif jt == NJT - 1:
                        plast[ca][1] = p_a
                        plast[cb][1] = p_b
                    return p_a, p_b

                # software pipeline: scores/exp emitted one jt ahead of PV
                # so the PE queue never stalls waiting on the exp latency
                p_next = scores_block(0)
                for jt in range(NJT):
                    p_a, p_b = p_next
                    if jt + 1 < NJT:
                        p_next = scores_block(jt + 1)
                    for m in range(2):
                        vslice = vvp[jt // 2][:, (jt % 2) * 256 + m * 128:
                                              (jt % 2) * 256 + m * 128 + 128]
                        nc.tensor.matmul(pv_ps[0][m][:], vslice, p_a[:],
                                         start=(jt == 0), stop=(jt == NJT - 1))
                        nc.tensor.matmul(pv_ps[1][m][:], vslice, p_b[:],
                                         start=(jt == 0), stop=(jt == NJT - 1))
                    if sc == 0:
                        if jt % 4 == 0 and 4 <= jt <= 24:
                            kproj(jt // 4 + 1)
                        if jt == 2:
                            qproj(2)
                        if jt == 6:
                            qproj(3)
                        if jt % 2 == 0 and jt < NJT - 2:
                            vvproj(jt // 2 + 1)
                    if jt == 0:
                        nc.vector.tensor_copy(lacc[ca][:], p_a[:])
                        nc.gpsimd.tensor_copy(lacc[cb][:], p_b[:])
                    elif jt < NJT - 2:
                        nc.vector.tensor_add(lacc[ca][:], lacc[ca][:], p_a[:])
                        nc.gpsimd.tensor_add(lacc[cb][:], lacc[cb][:], p_b[:])
                    if sc == 1 and jt == 6:
                        finalize(0)
                    if sc == 1 and jt == 20:
                        make_rx(2)
                        make_rx(3)
                if sc == 0:
                    make_rx(0)
                    make_rx(1)
                    # evacuate pv PSUM so super1 can reuse the banks
                    for ci in range(2):
                        for m in range(2):
                            o = resp.tile([128, IC], f32r, tag=f"osb{ci}_{m}",
                                          name=f"osb{ci}_{m}", bufs=1)
                            nc.vector.tensor_copy(o[:], pv_ps[ci][m][:])
                            osb[2 * sc + ci][m] = o
                else:
                    finalize(1, from_psum=pv_ps)

    nc.compile()
    return nc


def _prep_inputs(x, gn_gamma, gn_beta, wq, bq, wk, bk, wv, bv, wo, bo):
    f = np.float32
    constR = np.zeros((128, _RCOLS), f)
    wov = (wo.astype(f) @ wv.astype(f))
    for base, w in ((_RQ, wq), (_RK, wk), (_RO, wov)):
        wT = w.astype(f).T  # [c_in, c_out]
        for t in range(2):
            constR[:, base + 256 * t:base + 256 * t + 256] = \
                wT[128 * t:128 * (t + 1), :]
    constF = np.zeros((128, _FCOLS), f)
    gsel = np.zeros((2, 128, GROUPS), f)
    gselT = np.zeros((2, GROUPS, 128), f)
    for t in range(2):
        for p in range(128):
            g = (t * 128 + p) // CPG
            gsel[t, p, g] = 1.0
            gselT[t, g, p] = 1.0
    for t in range(2):
        constF[:, _FGS + 32 * t:_FGS + 32 * t + 32] = gsel[t]
        constF[0:GROUPS, _FGT + 128 * t:_FGT + 128 * t + 128] = gselT[t]
    wobvbo = (wo.astype(f) @ bv.astype(f) + bo.astype(f))
    vecs = (gn_gamma, gn_beta, bq, wobvbo)
    for i, v in enumerate(vecs):
        vv = v.astype(f).reshape(2, 128)
        for t in range(2):
            constF[:, _FVEC + 2 * i + t] = vv[t]
    constF[:, _FKC + 0] = -SHIFT
    constF[:, _FKC + 1] = EPS

    common = dict(constR=constR, constF=constF)
    xb = x.reshape(B, C, N).astype(f)
    in_maps = []
    for core in range(NCORES):
        bi, qh = core // 2, core % 2
        xcore = xb[bi]
        if qh:
            xcore = np.concatenate([xcore[:, NQ:], xcore[:, :NQ]], axis=1)
        in_maps.append(dict(x=np.ascontiguousarray(xcore.reshape(2, 128, N)),
                            **common))
    return in_maps


def _execute(inputs, trace=False, **kw):
    from concourse.bass_utils import run_bass_kernel_spmd
    if "nc" not in _cache:
        _cache["nc"] = _build()
    nc = _cache["nc"]
    in_maps = _prep_inputs(**inputs)
    res = run_bass_kernel_spmd(nc, in_maps, core_ids=list(range(NCORES)),
                               trace=trace, **kw)
    out = np.empty((B, C, N), np.float32)
    for core in range(NCORES):
        bi, qh = core // 2, core % 2
        chunk = res.results[core]["out"].reshape(C, NQ)
        out[bi, :, qh * NQ:(qh + 1) * NQ] = chunk
    return out.reshape(B, C, DD, HH, WW), res


def kernel(**inputs):
    out, _ = _execute(inputs, trace=False)
    return out



# revision 12
# speedup vs baseline: 1.0425x; 1.0425x over previous
"""AttnBlock3d on 8 TRN2 NeuronCores.

Sharding: 8 cores = 4 batches x 2 query-halves. Each core receives its
batch's full x (rotated so its query half is always voxels [0:2048] --
GroupNorm and the attention key-reduction are voxel-permutation
invariant, so all cores run an identical graph), computes GN stats +
QKV + full attention for its 2048 queries and writes a [2,128,2048]
channel-tiled chunk.

Math restructuring vs the reference (exact up to fp rounding):
- x is shipped as fp16 (halves the HBM load; fp16 keeps 11 mantissa
  bits so the scores stay accurate, unlike bf16).  The GN apply is
  folded into a per-channel scale of the MOVING x: xt = a*x with
  a = gamma*rstd; projections use the RAW f32r transposed weights as
  stationaries (mixed fp16 x bass-matmul).  The b-terms reduce to
  per-query score shifts (softmax-invariant, dropped) plus the q bias
  cq = Wq b + bq (tiny on-chip matmuls); k needs no bias at all.
- The output projection is fused into V: vv = (Wo Wv) xt (Wo@Wv
  precomputed on host) so PV directly produces the o-projected output;
  the remaining constant (WoWv b + Wo bv + bo) is added at the end.
- Scores are computed transposed (S^T = k^T q, f32r) so no transposes
  are needed; exp applies a constant shift (scores stay in [-97,97]).
  The softmax denominator accumulates P^T tiles on Vector/GpSimd,
  summed across partitions by a ones-matmul; normalization is applied
  at the very end (linearity).
- The x load is CHAINED (tiny guard-DMAs on the Sync queue serialize
  the piece triggers) so pieces arrive in order and the GN stats
  (Scalar: sum-accum; Vector: tensor_tensor_reduce sumsq) overlap the
  load instead of serializing after it.
- The attention loop processes TWO 512-query chunks per key-tile pass
  so each stationary tile serves 2 matmuls.
- The last super's softmax denominators + reciprocals are hoisted
  before the final PV matmuls; the closing normalize+residual is split
  Vector/GpSimd and the output DMA is merged to one [128,1024] write
  per channel-half, so almost nothing trails the last matmul.
- The DMA-load head warms the PE (HAM) with junk matmuls paced by x
  piece arrivals, and activation tables (Identity/Sqrt/Exp) are
  preloaded with dummies so no table load lands on the critical path.
"""

import sys

for _p in ("/opt/trn_rl_repo",):
    if _p not in sys.path:
        sys.path.append(_p)

import numpy as np

B, C, DD, HH, WW = 4, 256, 16, 16, 16
N = DD * HH * WW          # 4096 voxels
NQ = N // 2               # queries per core
GROUPS = 32
CPG = C // GROUPS         # channels per group
EPS = 1e-6
SHIFT = 60.0              # softmax constant shift
NCORES = 8
IC = 512                  # query chunk
NIC = NQ // IC            # 4 chunks
NJT = N // 128            # 32 key tiles
XC = 2048                 # x-load / GN piece (voxels)
NXC = N // XC             # 2 pieces per channel-half

# packed-constant column offsets (constR: f32r, constF: f32)
_RQ, _RK, _RO = 0, 512, 1024
_RCOLS = 1536
_FGS, _FGT, _FVEC, _FKC = 0, 64, 320, 328
_FCOLS = 330

CHAIN = 0  # serialize x piece DMAs via guard-DMAs on the Sync queue

_cache = {}


def _build():
    import concourse.bass as bass
    from concourse import bacc, mybir, tile

    f32 = mybir.dt.float32
    f32r = mybir.dt.float32r
    f16 = mybir.dt.float16
    bf16 = mybir.dt.bfloat16
    AF = mybir.ActivationFunctionType
    OP = mybir.AluOpType
    AX = mybir.AxisListType

    nc = bacc.Bacc("TRN2", target_bir_lowering=False, debug=False,
                   num_devices=NCORES)

    x_e = nc.dram_tensor("x", [2, 128, N], f16, kind="ExternalInput").ap()
    cR_e = nc.dram_tensor("constR", [128, _RCOLS], f16,
                          kind="ExternalInput").ap()
    cF_e = nc.dram_tensor("constF", [128, _FCOLS], f32,
                          kind="ExternalInput").ap()
    out_e = nc.dram_tensor("out", [2, 128, NQ], f32, kind="ExternalOutput").ap()

    with tile.TileContext(nc) as tc:
        with tc.tile_pool(name="big", bufs=1) as big, \
             tc.tile_pool(name="w", bufs=1) as wp, \
             tc.tile_pool(name="sm", bufs=2) as sm, \
             tc.tile_pool(name="pt", bufs=12) as ptp, \
             tc.tile_pool(name="res", bufs=2) as resp, \
             tc.tile_pool(name="psum", bufs=1, space="PSUM") as ps:

            # ---- on-chip constants (no DMA dependency) ----
            ones128 = wp.tile([128, 128], f32r, tag="ones", name="ones128")
            ones_h = wp.tile([128, 128], bf16, tag="onesh", name="ones_h")
            ones_f = wp.tile([128, 128], f32, tag="onesf", name="ones_f")
            ones16 = wp.tile([128, 128], f16, tag="ones16", name="ones16")
            nc.vector.memset(ones_f[:], 1.0)
            nc.vector.tensor_copy(ones128[:], ones_f[:])
            nc.vector.tensor_copy(ones_h[:], ones_f[:])
            nc.vector.tensor_copy(ones16[:], ones_f[:])
            dzi = sm.tile([GROUPS, 1], f32, tag="dzi", name="dzi")
            dzo = sm.tile([GROUPS, 1], f32, tag="dzo", name="dzo")
            nc.vector.memset(dzi[:], 0.25)
            # preload the activation tables off the critical path
            nc.scalar.activation(dzo[:], dzi[:], AF.Identity)
            nc.scalar.activation(dzo[:], dzi[:], AF.Sqrt)
            nc.scalar.activation(dzo[:], dzi[:], AF.Exp)

            # ---- chained x load: pieces arrive IN ORDER so GN stats
            # overlap the load.  Tiny guard-DMAs on the Sync queue make the
            # engine wait for a piece before triggering later pieces. ----
            xc = [[big.tile([128, XC], f16, tag=f"x{t}_{cx}", name=f"x{t}_{cx}")
                   for cx in range(NXC)] for t in range(2)]
            cR = wp.tile([128, _RCOLS], f16, tag="cR", name="cR")
            cF = wp.tile([128, _FCOLS], f32, tag="cF", name="cF")
            gdst = sm.tile([1, 16], f16, tag="gdst", name="gdst")

            def trig(t, cx):
                sl = slice(cx * XC, (cx + 1) * XC)
                nc.sync.dma_start(xc[t][cx][:], x_e[t, :, sl])

            def guard(t, cx):
                nc.sync.dma_start(gdst[:], xc[t][cx][0:1, 0:16])

            # piece order: (0,0) (1,0) (0,1) (1,1)
            if CHAIN:
                trig(0, 0)
                guard(0, 0)
                trig(1, 0)
                nc.sync.dma_start(cF[:], cF_e[:])
                guard(1, 0)
                trig(0, 1)
                trig(1, 1)
                guard(0, 1)
                nc.sync.dma_start(cR[:], cR_e[:])
            else:
                trig(0, 0)
                trig(1, 0)
                nc.sync.dma_start(cF[:], cF_e[:])
                trig(0, 1)
                trig(1, 1)
                nc.sync.dma_start(cR[:], cR_e[:])

            # raw transposed weights [c_in, c_out]; blocks (2t+m)
            wqT = [[cR[:, _RQ + 128 * (2 * t + m):_RQ + 128 * (2 * t + m) + 128]
                    for m in range(2)] for t in range(2)]
            wovT = [cR[:, _RO + 256 * t:_RO + 256 * t + 256] for t in range(2)]
            gsel = [cF[:, _FGS + 32 * t:_FGS + 32 * t + 32] for t in range(2)]
            gselT = [cF[0:GROUPS, _FGT + 128 * t:_FGT + 128 * t + 128]
                     for t in range(2)]
            gamma = [cF[:, _FVEC + 0 + t:_FVEC + 1 + t] for t in range(2)]
            beta = [cF[:, _FVEC + 2 + t:_FVEC + 3 + t] for t in range(2)]
            bq = [cF[:, _FVEC + 4 + t:_FVEC + 5 + t] for t in range(2)]
            wobvbo = [cF[:, _FVEC + 6 + t:_FVEC + 7 + t] for t in range(2)]
            kconst = cF[:, _FKC:_FKC + 2]

            # ---- GN stats overlap the chained load: per piece Scalar does
            # the sum (Identity + accum into a junk out) and Vector does the
            # sumsq (tensor_tensor_reduce); PE warms (HAM) pace on pieces ----
            sum4 = [sm.tile([128, NXC], f32, tag=f"sum4{t}", name=f"sum4{t}")
                    for t in range(2)]
            sq4 = [sm.tile([128, NXC], f32, tag=f"sq4{t}", name=f"sq4{t}")
                   for t in range(2)]
            stats = [sm.tile([128, 2], f32, tag=f"st{t}", name=f"st{t}")
                     for t in range(2)]
            for cx in range(NXC):
                for t in range(2):
                    sj = sm.tile([128, XC], f16, tag="sjunk", name="sjunk",
                                 bufs=1)
                    nc.scalar.activation(sj[:], xc[t][cx][:], AF.Identity,
                                         accum_out=sum4[t][:, cx:cx + 1])
                    vj = sm.tile([128, XC], f16, tag="vjunk", name="vjunk",
                                 bufs=1)
                    nc.scalar.activation(vj[:], xc[t][cx][:], AF.Square,
                                         accum_out=sq4[t][:, cx:cx + 1])
                    for r in range(3):
                        w_ps = ps.tile([128, IC], f32, tag="s", name="warm",
                                       bufs=4)
                        nc.tensor.matmul(w_ps[:], ones16,
                                         xc[t][cx][:, (r % 4) * 512:
                                                    (r % 4) * 512 + 512],
                                         start=True, stop=True)
            # dense warm burst on the last piece to flip HAM before the
            # real stream starts
            for r in range(10):
                t, co = r % 2, (r % 4) * 512
                w_ps = ps.tile([128, IC], f32, tag="s", name="warm", bufs=4)
                nc.tensor.matmul(w_ps[:], ones16,
                                 xc[t][NXC - 1][:, co:co + IC],
                                 start=True, stop=True)

            def warmmm(n):
                for r in range(n):
                    t, co = r % 2, (r % 4) * 512
                    w_ps = ps.tile([128, IC], f32, tag="s", name="warm",
                                   bufs=4)
                    nc.tensor.matmul(w_ps[:], ones16,
                                     xc[t][0][:, co:co + IC],
                                     start=True, stop=True)

            # ---- stats combine -> per-channel a, b ----
            for t in range(2):
                nc.vector.reduce_sum(stats[t][:, 0:1], sum4[t][:], axis=AX.X)
                nc.vector.reduce_sum(stats[t][:, 1:2], sq4[t][:], axis=AX.X)
            g_ps = ps.tile([GROUPS, 2], f32, tag="s", name="g_ps", bufs=4)
            for t in range(2):
                nc.tensor.matmul(g_ps[:], gsel[t], stats[t][:],
                                 start=(t == 0), stop=(t == 1))
            for t in range(2):
                f_ps = ps.tile([128, 2], f32, tag="s", name="fill", bufs=4)
                nc.tensor.matmul(f_ps[:], ones_f, stats[t][:],
                                 start=True, stop=True)
                warmmm(2)
            gstats = sm.tile([GROUPS, 2], f32, tag="gstats", name="gstats")
            var = sm.tile([GROUPS, 1], f32, tag="gvar", name="gvar")
            stdt = sm.tile([GROUPS, 1], f32, tag="gstd", name="gstd")
            inv = 1.0 / (CPG * N)
            nc.vector.tensor_scalar_mul(gstats[:, 0:2], g_ps[:, 0:2], inv)
            # negvar = mean*mean - ex2 ; std = sqrt(-negvar + eps)
            nc.vector.scalar_tensor_tensor(var[:], gstats[:, 0:1],
                                           gstats[:, 0:1], gstats[:, 1:2],
                                           op0=OP.mult, op1=OP.subtract)
            nc.scalar.activation(stdt[:], var[:], AF.Sqrt,
                                 bias=kconst[0:GROUPS, 1:2], scale=-1.0)
            nc.vector.reciprocal_approx_fast(gstats[:, 1:2], stdt[:])
            ab = [sm.tile([128, 2], f32, tag=f"ab{t}", name=f"ab{t}")
                  for t in range(2)]
            bcol = [sm.tile([128, 2], f16, tag=f"bcol{t}", name=f"bcol{t}")
                    for t in range(2)]
            for t in range(2):
                bc_ps = ps.tile([128, 2], f32, tag="s", name="bc_ps", bufs=4)
                nc.tensor.matmul(bc_ps[:], gselT[t], gstats[:],
                                 start=True, stop=True)
                # a = rstd*gamma ; b = beta - mean*a
                nc.vector.tensor_mul(ab[t][:, 0:1], bc_ps[:, 1:2], gamma[t])
                nc.vector.tensor_mul(ab[t][:, 1:2], bc_ps[:, 0:1], ab[t][:, 0:1])
                nc.vector.tensor_sub(ab[t][:, 1:2], beta[t], ab[t][:, 1:2])
                nc.vector.tensor_copy(bcol[t][:, 0:1], ab[t][:, 1:2])
                nc.vector.tensor_copy(bcol[t][:, 1:2], ab[t][:, 1:2])

            f_ps = ps.tile([128, 2], f32, tag="s", name="fill", bufs=4)
            nc.tensor.matmul(f_ps[:], ones_f[0:GROUPS, :], gstats[:],
                             start=True, stop=True)
            warmmm(3)

            # ---- scaled moving x: xt = a * x (fp16), chunk 0 first ----
            xt = [[big.tile([128, XC], f16, tag=f"xt{t}_{cx}",
                            name=f"xt{t}_{cx}") for cx in range(NXC)]
                  for t in range(2)]
            for cx in range(NXC):
                for t in range(2):
                    nc.vector.tensor_scalar_mul(xt[t][cx][:], xc[t][cx][:],
                                                ab[t][:, 0:1])

            # ---- on-chip bias columns: cq = Wq b + bq (per attention-channel
            # tile), bfin = WoWv b + (Wo bv + bo) ----
            cqc = [sm.tile([128, 1], f32, tag=f"cq{m}", name=f"cq{m}")
                   for m in range(2)]
            bfin = [sm.tile([128, 1], f32, tag=f"bf{m}", name=f"bf{m}")
                    for m in range(2)]
            for m in range(2):
                c_ps = ps.tile([128, 2], f32, tag="s", name="c_ps", bufs=4)
                for t in range(2):
                    nc.tensor.matmul(c_ps[:], wqT[t][m], bcol[t][:],
                                     start=(t == 0), stop=(t == 1))
                nc.vector.tensor_scalar_add(cqc[m][:], c_ps[:, 0:1], bq[m])
            for m in range(2):
                c_ps = ps.tile([128, 2], f32, tag="s", name="c_ps", bufs=4)
                for t in range(2):
                    nc.tensor.matmul(c_ps[:], wovT[t][:, m * 128:m * 128 + 128],
                                     bcol[t][:], start=(t == 0), stop=(t == 1))
                nc.vector.tensor_scalar_add(bfin[m][:], c_ps[:, 0:1], wobvbo[m])
            warmmm(2)

            # ---- projections on scaled x ----
            qt = [[big.tile([128, IC], f32r, tag=f"q{e}_{f}", name=f"q{e}_{f}")
                   for f in range(NIC)] for e in range(2)]
            kt = [[big.tile([128, 512], f32r, tag=f"k{e}_{f}", name=f"k{e}_{f}")
                   for f in range(N // 512)] for e in range(2)]
            # vv pairs: [vv(2u) | vv(2u+1)], each [128 keys, 256 ch] bf16
            vvp = [big.tile([128, 512], bf16, tag=f"vv{u}", name=f"vv{u}")
                   for u in range(NJT // 2)]

            def qproj(f):
                for e in range(2):
                    q_ps = ps.tile([128, IC], f32, tag="s", name="q_ps",
                                   bufs=4)
                    for t in range(2):
                        nc.tensor.matmul(
                            q_ps[:],
                            cR[:, _RQ + 256 * t + 128 * e:
                               _RQ + 256 * t + 128 * e + 128],
                            xt[t][0][:, f * 512:(f + 1) * 512],
                            start=(t == 0), stop=(t == 1))
                    nc.vector.tensor_scalar_add(qt[e][f][:], q_ps[:], cqc[e])

            def kproj(f):
                for e in range(2):
                    k_ps = ps.tile([128, 512], f32, tag="s", name="k_ps",
                                   bufs=4)
                    for t in range(2):
                        nc.tensor.matmul(
                            k_ps[:],
                            cR[:, _RK + 256 * t + 128 * e:
                               _RK + 256 * t + 128 * e + 128],
                            xt[t][f // 4][:, (f % 4) * 512:(f % 4 + 1) * 512],
                            start=(t == 0), stop=(t == 1))
                    nc.vector.tensor_copy(kt[e][f][:], k_ps[:])

            def vvproj(u):
                v_ps = ps.tile([128, 512], f32, tag="s", name="v_ps", bufs=4)
                for jj in range(2):
                    jt = 2 * u + jj
                    for t in range(2):
                        nc.tensor.matmul(
                            v_ps[:, jj * 256:jj * 256 + 256],
                            xt[t][jt // 16][:, (jt % 16) * 128:
                                            (jt % 16) * 128 + 128],
                            wovT[t], start=(t == 0), stop=(t == 1))
                nc.vector.tensor_copy(vvp[u][:], v_ps[:])

            kproj(0)
            kproj(1)
            qproj(0)
            qproj(1)
            vvproj(0)

            # ---- attention: 2 supers x 2 chunks x 32 key tiles ----
            lacc = [resp.tile([128, IC], f32r, tag=f"lacc{c}", name=f"lacc{c}",
                              bufs=1) for c in range(NIC)]
            osb = [[None, None] for c in range(NIC)]
            rx = [[None, None] for c in range(NIC)]
            plast = {}
            rbt = {}

            def make_rx(c):
                for m in range(2):
                    r = resp.tile([128, IC], f32, tag=f"rx{c}_{m}",
                                  name=f"rx{c}_{m}", bufs=1)
                    nc.vector.tensor_scalar_add(
                        r[:], xc[m][0][:, c * IC:(c + 1) * IC], bfin[m])
                    rx[c][m] = r

            def denom(c):
                # softmax denominator for chunk c -> reciprocal rbt[c]
                lbc_ps = ps.tile([128, IC], f32, tag="s", name="lbc_ps",
                                 bufs=4)
                nc.tensor.matmul(lbc_ps[:], ones128, lacc[c][:],
                                 start=True, stop=False)
                p30, p31 = plast[c]
                nc.tensor.matmul(lbc_ps[:], ones_h, p30[:],
                                 start=False, stop=False)
                nc.tensor.matmul(lbc_ps[:], ones_h, p31[:],
                                 start=False, stop=True)
                rb = resp.tile([128, IC], f32, tag=f"rb{c}", name=f"rb{c}",
                               bufs=1)
                nc.vector.reciprocal_approx_fast(rb[:], lbc_ps[:])
                rbt[c] = rb

            def finalize0(sc):
                # deferred finalize for super-0 chunks (hidden mid-stream):
                # GpSimd adds, merged [128,1024] out DMA per channel-half
                denom(2 * sc)
                denom(2 * sc + 1)
                for m in range(2):
                    res = resp.tile([128, 1024], f32, tag=f"res0_{m}",
                                    name=f"res0_{m}", bufs=1)
                    for ci in range(2):
                        c = 2 * sc + ci
                        scaled = resp.tile([128, IC], f32, tag="scaled",
                                           name="scaled")
                        nc.vector.tensor_mul(scaled[:], osb[c][m][:],
                                             rbt[c][:])
                        nc.gpsimd.tensor_add(res[:, ci * IC:(ci + 1) * IC],
                                             scaled[:], rx[c][m][:])
                    nc.sync.dma_start(out_e[m, :, 2 * sc * IC:
                                            (2 * sc + 2) * IC], res[:])

            for sc in range(2):
                ca, cb = 2 * sc, 2 * sc + 1
                pv_ps = [[ps.tile([128, IC], f32, tag=f"pv{ci}_{m}",
                                  name=f"pv{ci}_{m}", bufs=1)
                          for m in range(2)] for ci in range(2)]

                def scores_block(jt):
                    s_a = ps.tile([128, IC], f32, tag="s", name="s_a", bufs=4)
                    s_b = ps.tile([128, IC], f32, tag="s", name="s_b", bufs=4)
                    for e in range(2):
                        ktile = kt[e][jt // 4][:, (jt % 4) * 128:(jt % 4 + 1) * 128]
                        nc.tensor.matmul(s_a[:], ktile, qt[e][ca][:],
                                         start=(e == 0), stop=(e == 1))
                        nc.tensor.matmul(s_b[:], ktile, qt[e][cb][:],
                                         start=(e == 0), stop=(e == 1))
                    ptag = "pt" if jt < NJT - 2 else "pfin"
                    pbufs = {} if jt < NJT - 2 else {"bufs": 4}
                    p_a = ptp.tile([128, IC], bf16, tag=ptag, name=ptag,
                                   **pbufs)
                    nc.scalar.activation(p_a[:], s_a[:], AF.Exp,
                                         bias=kconst[:, 0:1])
                    p_b = ptp.tile([128, IC], bf16, tag=ptag, name=ptag,
                                   **pbufs)
                    nc.scalar.activation(p_b[:], s_b[:], AF.Exp,
                                         bias=kconst[:, 0:1])
                    if jt == NJT - 2:
                        plast[ca] = [p_a, None]
                        plast[cb] = [p_b, None]
                    elif jt == NJT - 1:
                        plast[ca][1] = p_a
                        plast[cb][1] = p_b
                    return p_a, p_b

                # software pipeline: scores/exp emitted one jt ahead of PV
                # so the PE queue never stalls waiting on the exp latency
                p_next = scores_block(0)
                for jt in range(NJT):
                    p_a, p_b = p_next
                    if jt + 1 < NJT:
                        p_next = scores_block(jt + 1)
                    if sc == 1 and jt == NJT - 1:
                        # hoist the denominators + reciprocals ahead of the
                        # final PV matmuls so only mul/add/DMA trail the PE
                        denom(ca)
                        denom(cb)
                    for m in range(2):
                        vslice = vvp[jt // 2][:, (jt % 2) * 256 + m * 128:
                                              (jt % 2) * 256 + m * 128 + 128]
                        nc.tensor.matmul(pv_ps[0][m][:], vslice, p_a[:],
                                         start=(jt == 0), stop=(jt == NJT - 1))
                        nc.tensor.matmul(pv_ps[1][m][:], vslice, p_b[:],
                                         start=(jt == 0), stop=(jt == NJT - 1))
                    if sc == 0:
                        if jt % 4 == 0 and 4 <= jt <= 24:
                            kproj(jt // 4 + 1)
                        if jt == 2:
                            qproj(2)
                        if jt == 6:
                            qproj(3)
                        if jt % 2 == 0 and jt < NJT - 2:
                            vvproj(jt // 2 + 1)
                    if jt == 0:
                        nc.vector.tensor_copy(lacc[ca][:], p_a[:])
                        nc.gpsimd.tensor_copy(lacc[cb][:], p_b[:])
                    elif jt < NJT - 2:
                        nc.vector.tensor_add(lacc[ca][:], lacc[ca][:], p_a[:])
                        nc.gpsimd.tensor_add(lacc[cb][:], lacc[cb][:], p_b[:])
                    if sc == 1 and jt == 6:
                        finalize0(0)
                    if sc == 1 and jt == 20:
                        make_rx(2)
                        make_rx(3)
                if sc == 0:
                    make_rx(0)
                    make_rx(1)
                    # evacuate pv PSUM so super1 can reuse the banks
                    for ci in range(2):
                        for m in range(2):
                            o = resp.tile([128, IC], f32r, tag=f"osb{ci}_{m}",
                                          name=f"osb{ci}_{m}", bufs=1)
                            nc.vector.tensor_copy(o[:], pv_ps[ci][m][:])
                            osb[2 * sc + ci][m] = o
                else:
                    # closing finalize: Vector does the PSUM-reading muls
                    # (GpSimd cannot access PSUM) + the m=0 adds; GpSimd does
                    # the m=1 adds; merged [128,1024] DMA per channel-half
                    res1 = [resp.tile([128, 1024], f32, tag=f"res1_{m}",
                                      name=f"res1_{m}", bufs=1)
                            for m in range(2)]
                    for ci in range(2):
                        c = 2 * sc + ci
                        scl = [resp.tile([128, IC], f32, tag=f"scl{ci}_{m}",
                                         name=f"scl{ci}_{m}", bufs=1)
                               for m in range(2)]
                        nc.vector.tensor_mul(scl[0][:], pv_ps[ci][0][:],
                                             rbt[c][:])
                        nc.vector.tensor_mul(scl[1][:], pv_ps[ci][1][:],
                                             rbt[c][:])
                        nc.vector.tensor_add(res1[0][:, ci * IC:(ci + 1) * IC],
                                             scl[0][:], rx[c][0][:])
                        nc.gpsimd.tensor_add(res1[1][:, ci * IC:(ci + 1) * IC],
                                             scl[1][:], rx[c][1][:])
                    for m in range(2):
                        nc.sync.dma_start(out_e[m, :, 2 * sc * IC:
                                                (2 * sc + 2) * IC], res1[m][:])

    nc.compile()
    return nc


def _prep_inputs(x, gn_gamma, gn_beta, wq, bq, wk, bk, wv, bv, wo, bo):
    f = np.float32
    constR = np.zeros((128, _RCOLS), np.float16)
    wov = (wo.astype(f) @ wv.astype(f))
    for base, w in ((_RQ, wq), (_RK, wk), (_RO, wov)):
        wT = w.astype(f).T  # [c_in, c_out]
        for t in range(2):
            constR[:, base + 256 * t:base + 256 * t + 256] = \
                wT[128 * t:128 * (t + 1), :]
    constF = np.zeros((128, _FCOLS), f)
    gsel = np.zeros((2, 128, GROUPS), f)
    gselT = np.zeros((2, GROUPS, 128), f)
    for t in range(2):
        for p in range(128):
            g = (t * 128 + p) // CPG
            gsel[t, p, g] = 1.0
            gselT[t, g, p] = 1.0
    for t in range(2):
        constF[:, _FGS + 32 * t:_FGS + 32 * t + 32] = gsel[t]
        constF[0:GROUPS, _FGT + 128 * t:_FGT + 128 * t + 128] = gselT[t]
    wobvbo = (wo.astype(f) @ bv.astype(f) + bo.astype(f))
    vecs = (gn_gamma, gn_beta, bq, wobvbo)
    for i, v in enumerate(vecs):
        vv = v.astype(f).reshape(2, 128)
        for t in range(2):
            constF[:, _FVEC + 2 * i + t] = vv[t]
    constF[:, _FKC + 0] = -SHIFT
    constF[:, _FKC + 1] = EPS

    common = dict(constR=constR, constF=constF)
    xb = x.reshape(B, C, N).astype(np.float16)
    in_maps = []
    for core in range(NCORES):
        bi, qh = core // 2, core % 2
        xcore = xb[bi]
        if qh:
            xcore = np.concatenate([xcore[:, NQ:], xcore[:, :NQ]], axis=1)
        in_maps.append(dict(x=np.ascontiguousarray(xcore.reshape(2, 128, N)),
                            **common))
    return in_maps


def _execute(inputs, trace=False, **kw):
    from concourse.bass_utils import run_bass_kernel_spmd
    if "nc" not in _cache:
        _cache["nc"] = _build()
    nc = _cache["nc"]
    in_maps = _prep_inputs(**inputs)
    res = run_bass_kernel_spmd(nc, in_maps, core_ids=list(range(NCORES)),
                               trace=trace, **kw)
    out = np.empty((B, C, N), np.float32)
    for core in range(NCORES):
        bi, qh = core // 2, core % 2
        chunk = res.results[core]["out"].reshape(C, NQ)
        out[bi, :, qh * NQ:(qh + 1) * NQ] = chunk
    return out.reshape(B, C, DD, HH, WW), res


def kernel(**inputs):
    out, _ = _execute(inputs, trace=False)
    return out


# revision 13
# speedup vs baseline: 1.0725x; 1.0288x over previous
"""AttnBlock3d on 8 TRN2 NeuronCores.

Sharding: 8 cores = 4 batches x 2 query-halves. Each core receives its
batch's full x (rotated so its query half is always voxels [0:2048] --
GroupNorm and the attention key-reduction are voxel-permutation
invariant, so all cores run an identical graph), computes GN stats +
QKV + full attention for its 2048 queries and writes a [2,128,2048]
channel-tiled chunk.

Math restructuring vs the reference (exact up to fp rounding):
- x is shipped as fp16 (halves the HBM load; fp16 keeps 11 mantissa
  bits so the scores stay accurate, unlike bf16).  The GN apply is
  folded into a per-channel scale of the MOVING x: xt = a*x with
  a = gamma*rstd; projections use the RAW f32r transposed weights as
  stationaries (mixed fp16 x bass-matmul).  The b-terms reduce to
  per-query score shifts (softmax-invariant, dropped) plus the q bias
  cq = Wq b + bq (tiny on-chip matmuls); k needs no bias at all.
- The output projection is fused into V: vv = (Wo Wv) xt (Wo@Wv
  precomputed on host) so PV directly produces the o-projected output;
  the remaining constant (WoWv b + Wo bv + bo) is added at the end.
- Scores are computed transposed (S^T = k^T q, f32r) so no transposes
  are needed; exp applies a constant shift (scores stay in [-97,97]).
  The softmax denominator accumulates P^T tiles on Vector/GpSimd,
  summed across partitions by a ones-matmul; normalization is applied
  at the very end (linearity).
- The x load is CHAINED (tiny guard-DMAs on the Sync queue serialize
  the piece triggers) so pieces arrive in order and the GN stats
  (Scalar: sum-accum; Vector: tensor_tensor_reduce sumsq) overlap the
  load instead of serializing after it.
- The attention loop processes TWO 512-query chunks per key-tile pass
  so each stationary tile serves 2 matmuls.
- The last super's softmax denominators + reciprocals are hoisted
  before the final PV matmuls; the closing normalize+residual is split
  Vector/GpSimd and the output DMA is merged to one [128,1024] write
  per channel-half, so almost nothing trails the last matmul.
- The DMA-load head warms the PE (HAM) with junk matmuls paced by x
  piece arrivals, and activation tables (Identity/Sqrt/Exp) are
  preloaded with dummies so no table load lands on the critical path.
"""

import sys

for _p in ("/opt/trn_rl_repo",):
    if _p not in sys.path:
        sys.path.append(_p)

import numpy as np

B, C, DD, HH, WW = 4, 256, 16, 16, 16
N = DD * HH * WW          # 4096 voxels
NQ = N // 2               # queries per core
GROUPS = 32
CPG = C // GROUPS         # channels per group
EPS = 1e-6
SHIFT = 60.0              # softmax constant shift
NCORES = 8
IC = 512                  # query chunk
NIC = NQ // IC            # 4 chunks
NJT = N // 128            # 32 key tiles
XC = 2048                 # x-load / GN piece (voxels)
NXC = N // XC             # 2 pieces per channel-half

# packed-constant column offsets (constR: f32r, constF: f32)
_RQ, _RK, _RO = 0, 512, 1024
_RCOLS = 1536
_FGS, _FGT, _FVEC, _FKC = 0, 64, 320, 328
_FCOLS = 330

CHAIN = 1  # serialize x piece DMAs via guard-DMAs on the Sync queue

_cache = {}


def _build():
    import concourse.bass as bass
    from concourse import bacc, mybir, tile

    f32 = mybir.dt.float32
    f32r = mybir.dt.float32r
    f16 = mybir.dt.float16
    bf16 = mybir.dt.bfloat16
    AF = mybir.ActivationFunctionType
    OP = mybir.AluOpType
    AX = mybir.AxisListType

    nc = bacc.Bacc("TRN2", target_bir_lowering=False, debug=False,
                   num_devices=NCORES)

    x_e = nc.dram_tensor("x", [2, 128, N], f16, kind="ExternalInput").ap()
    cR_e = nc.dram_tensor("constR", [128, _RCOLS], f16,
                          kind="ExternalInput").ap()
    cF_e = nc.dram_tensor("constF", [128, _FCOLS], f32,
                          kind="ExternalInput").ap()
    out_e = nc.dram_tensor("out", [2, 128, NQ], f32, kind="ExternalOutput").ap()

    with tile.TileContext(nc) as tc:
        with tc.tile_pool(name="big", bufs=1) as big, \
             tc.tile_pool(name="w", bufs=1) as wp, \
             tc.tile_pool(name="sm", bufs=2) as sm, \
             tc.tile_pool(name="pt", bufs=12) as ptp, \
             tc.tile_pool(name="res", bufs=2) as resp, \
             tc.tile_pool(name="psum", bufs=1, space="PSUM") as ps:

            # ---- on-chip constants (no DMA dependency) ----
            ones128 = wp.tile([128, 128], f32r, tag="ones", name="ones128")
            ones_h = wp.tile([128, 128], bf16, tag="onesh", name="ones_h")
            ones_f = wp.tile([128, 128], f32, tag="onesf", name="ones_f")
            ones16 = wp.tile([128, 128], f16, tag="ones16", name="ones16")
            nc.vector.memset(ones_f[:], 1.0)
            nc.vector.tensor_copy(ones128[:], ones_f[:])
            nc.vector.tensor_copy(ones_h[:], ones_f[:])
            nc.vector.tensor_copy(ones16[:], ones_f[:])
            dzi = sm.tile([GROUPS, 1], f32, tag="dzi", name="dzi")
            dzo = sm.tile([GROUPS, 1], f32, tag="dzo", name="dzo")
            nc.vector.memset(dzi[:], 0.25)
            # preload the activation tables off the critical path
            nc.scalar.activation(dzo[:], dzi[:], AF.Identity)
            nc.scalar.activation(dzo[:], dzi[:], AF.Sqrt)
            nc.scalar.activation(dzo[:], dzi[:], AF.Exp)

            # ---- chained x load: pieces arrive IN ORDER so GN stats
            # overlap the load.  Tiny guard-DMAs on the Sync queue make the
            # engine wait for a piece before triggering later pieces. ----
            xc = [[big.tile([128, XC], f16, tag=f"x{t}_{cx}", name=f"x{t}_{cx}")
                   for cx in range(NXC)] for t in range(2)]
            cR = wp.tile([128, _RCOLS], f16, tag="cR", name="cR")
            cF = wp.tile([128, _FCOLS], f32, tag="cF", name="cF")
            gdst = sm.tile([1, 16], f16, tag="gdst", name="gdst")

            def trig(t, cx):
                sl = slice(cx * XC, (cx + 1) * XC)
                nc.sync.dma_start(xc[t][cx][:], x_e[t, :, sl])

            def guard(t, cx):
                nc.sync.dma_start(gdst[:], xc[t][cx][0:1, 0:16])

            # piece order: (0,0) (1,0) (0,1) (1,1)
            if CHAIN:
                trig(0, 0)
                guard(0, 0)
                trig(1, 0)
                nc.sync.dma_start(cF[:], cF_e[:])
                guard(1, 0)
                trig(0, 1)
                trig(1, 1)
                guard(0, 1)
                nc.sync.dma_start(cR[:], cR_e[:])
            else:
                trig(0, 0)
                trig(1, 0)
                nc.sync.dma_start(cF[:], cF_e[:])
                trig(0, 1)
                trig(1, 1)
                nc.sync.dma_start(cR[:], cR_e[:])

            # raw transposed weights [c_in, c_out]; blocks (2t+m)
            wqT = [[cR[:, _RQ + 128 * (2 * t + m):_RQ + 128 * (2 * t + m) + 128]
                    for m in range(2)] for t in range(2)]
            wovT = [cR[:, _RO + 256 * t:_RO + 256 * t + 256] for t in range(2)]
            gsel = [cF[:, _FGS + 32 * t:_FGS + 32 * t + 32] for t in range(2)]
            gselT = [cF[0:GROUPS, _FGT + 128 * t:_FGT + 128 * t + 128]
                     for t in range(2)]
            gamma = [cF[:, _FVEC + 0 + t:_FVEC + 1 + t] for t in range(2)]
            beta = [cF[:, _FVEC + 2 + t:_FVEC + 3 + t] for t in range(2)]
            bq = [cF[:, _FVEC + 4 + t:_FVEC + 5 + t] for t in range(2)]
            wobvbo = [cF[:, _FVEC + 6 + t:_FVEC + 7 + t] for t in range(2)]
            kconst = cF[:, _FKC:_FKC + 2]

            # ---- GN stats overlap the chained load: per piece Scalar does
            # the sum (Identity + accum into a junk out) and Vector does the
            # sumsq (tensor_tensor_reduce); PE warms (HAM) pace on pieces ----
            sum4 = [sm.tile([128, NXC], f32, tag=f"sum4{t}", name=f"sum4{t}")
                    for t in range(2)]
            sq4 = [sm.tile([128, NXC], f32, tag=f"sq4{t}", name=f"sq4{t}")
                   for t in range(2)]
            stats = [sm.tile([128, 2], f32, tag=f"st{t}", name=f"st{t}")
                     for t in range(2)]
            for cx in range(NXC):
                for t in range(2):
                    sj = sm.tile([128, XC], f16, tag="sjunk", name="sjunk",
                                 bufs=1)
                    nc.scalar.activation(sj[:], xc[t][cx][:], AF.Identity,
                                         accum_out=sum4[t][:, cx:cx + 1])
                    vj = sm.tile([128, XC], f16, tag="vjunk", name="vjunk",
                                 bufs=1)
                    nc.vector.tensor_mul(vj[:], xc[t][cx][:], xc[t][cx][:])
                    nc.vector.reduce_sum(sq4[t][:, cx:cx + 1], vj[:],
                                         axis=AX.X)
                    for r in range(3):
                        w_ps = ps.tile([128, IC], f32, tag="s", name="warm",
                                       bufs=4)
                        nc.tensor.matmul(w_ps[:], ones16,
                                         xc[t][cx][:, (r % 4) * 512:
                                                    (r % 4) * 512 + 512],
                                         start=True, stop=True)
            # dense warm burst on the last piece to flip HAM before the
            # real stream starts
            for r in range(10):
                t, co = r % 2, (r % 4) * 512
                w_ps = ps.tile([128, IC], f32, tag="s", name="warm", bufs=4)
                nc.tensor.matmul(w_ps[:], ones16,
                                 xc[t][NXC - 1][:, co:co + IC],
                                 start=True, stop=True)

            def warmmm(n):
                for r in range(n):
                    t, co = r % 2, (r % 4) * 512
                    w_ps = ps.tile([128, IC], f32, tag="s", name="warm",
                                   bufs=4)
                    nc.tensor.matmul(w_ps[:], ones16,
                                     xc[t][0][:, co:co + IC],
                                     start=True, stop=True)

            # ---- stats combine -> per-channel a, b ----
            for t in range(2):
                nc.vector.reduce_sum(stats[t][:, 0:1], sum4[t][:], axis=AX.X)
                nc.vector.reduce_sum(stats[t][:, 1:2], sq4[t][:], axis=AX.X)
            g_ps = ps.tile([GROUPS, 2], f32, tag="s", name="g_ps", bufs=4)
            for t in range(2):
                nc.tensor.matmul(g_ps[:], gsel[t], stats[t][:],
                                 start=(t == 0), stop=(t == 1))
            for t in range(2):
                f_ps = ps.tile([128, 2], f32, tag="s", name="fill", bufs=4)
                nc.tensor.matmul(f_ps[:], ones_f, stats[t][:],
                                 start=True, stop=True)
                warmmm(2)
            gstats = sm.tile([GROUPS, 2], f32, tag="gstats", name="gstats")
            var = sm.tile([GROUPS, 1], f32, tag="gvar", name="gvar")
            stdt = sm.tile([GROUPS, 1], f32, tag="gstd", name="gstd")
            inv = 1.0 / (CPG * N)
            nc.vector.tensor_scalar_mul(gstats[:, 0:2], g_ps[:, 0:2], inv)
            # negvar = mean*mean - ex2 ; std = sqrt(-negvar + eps)
            nc.vector.scalar_tensor_tensor(var[:], gstats[:, 0:1],
                                           gstats[:, 0:1], gstats[:, 1:2],
                                           op0=OP.mult, op1=OP.subtract)
            nc.scalar.activation(stdt[:], var[:], AF.Sqrt,
                                 bias=kconst[0:GROUPS, 1:2], scale=-1.0)
            nc.vector.reciprocal_approx_fast(gstats[:, 1:2], stdt[:])
            ab = [sm.tile([128, 2], f32, tag=f"ab{t}", name=f"ab{t}")
                  for t in range(2)]
            bcol = [sm.tile([128, 2], f16, tag=f"bcol{t}", name=f"bcol{t}")
                    for t in range(2)]
            for t in range(2):
                bc_ps = ps.tile([128, 2], f32, tag="s", name="bc_ps", bufs=4)
                nc.tensor.matmul(bc_ps[:], gselT[t], gstats[:],
                                 start=True, stop=True)
                # a = rstd*gamma ; b = beta - mean*a
                nc.vector.tensor_mul(ab[t][:, 0:1], bc_ps[:, 1:2], gamma[t])
                nc.vector.tensor_mul(ab[t][:, 1:2], bc_ps[:, 0:1], ab[t][:, 0:1])
                nc.vector.tensor_sub(ab[t][:, 1:2], beta[t], ab[t][:, 1:2])
                nc.vector.tensor_copy(bcol[t][:, 0:1], ab[t][:, 1:2])
                nc.vector.tensor_copy(bcol[t][:, 1:2], ab[t][:, 1:2])

            f_ps = ps.tile([128, 2], f32, tag="s", name="fill", bufs=4)
            nc.tensor.matmul(f_ps[:], ones_f[0:GROUPS, :], gstats[:],
                             start=True, stop=True)
            warmmm(3)

            # ---- scaled moving x: xt = a * x (fp16), chunk 0 first ----
            xt = [[big.tile([128, XC], f16, tag=f"xt{t}_{cx}",
                            name=f"xt{t}_{cx}") for cx in range(NXC)]
                  for t in range(2)]
            for cx in range(NXC):
                for t in range(2):
                    nc.vector.tensor_scalar_mul(xt[t][cx][:], xc[t][cx][:],
                                                ab[t][:, 0:1])

            # ---- on-chip bias columns: cq = Wq b + bq (per attention-channel
            # tile), bfin = WoWv b + (Wo bv + bo) ----
            cqc = [sm.tile([128, 1], f32, tag=f"cq{m}", name=f"cq{m}")
                   for m in range(2)]
            bfin = [sm.tile([128, 1], f32, tag=f"bf{m}", name=f"bf{m}")
                    for m in range(2)]
            for m in range(2):
                c_ps = ps.tile([128, 2], f32, tag="s", name="c_ps", bufs=4)
                for t in range(2):
                    nc.tensor.matmul(c_ps[:], wqT[t][m], bcol[t][:],
                                     start=(t == 0), stop=(t == 1))
                nc.vector.tensor_scalar_add(cqc[m][:], c_ps[:, 0:1], bq[m])
            for m in range(2):
                c_ps = ps.tile([128, 2], f32, tag="s", name="c_ps", bufs=4)
                for t in range(2):
                    nc.tensor.matmul(c_ps[:], wovT[t][:, m * 128:m * 128 + 128],
                                     bcol[t][:], start=(t == 0), stop=(t == 1))
                nc.vector.tensor_scalar_add(bfin[m][:], c_ps[:, 0:1], wobvbo[m])
            warmmm(2)

            # ---- projections on scaled x ----
            qt = [[big.tile([128, IC], f32r, tag=f"q{e}_{f}", name=f"q{e}_{f}")
                   for f in range(NIC)] for e in range(2)]
            kt = [[big.tile([128, 512], f32r, tag=f"k{e}_{f}", name=f"k{e}_{f}")
                   for f in range(N // 512)] for e in range(2)]
            # vv pairs: [vv(2u) | vv(2u+1)], each [128 keys, 256 ch] bf16
            vvp = [big.tile([128, 512], bf16, tag=f"vv{u}", name=f"vv{u}")
                   for u in range(NJT // 2)]

            def qproj(f):
                for e in range(2):
                    q_ps = ps.tile([128, IC], f32, tag="s", name="q_ps",
                                   bufs=4)
                    for t in range(2):
                        nc.tensor.matmul(
                            q_ps[:],
                            cR[:, _RQ + 256 * t + 128 * e:
                               _RQ + 256 * t + 128 * e + 128],
                            xt[t][0][:, f * 512:(f + 1) * 512],
                            start=(t == 0), stop=(t == 1))
                    nc.vector.tensor_scalar_add(qt[e][f][:], q_ps[:], cqc[e])

            def kproj(f):
                for e in range(2):
                    k_ps = ps.tile([128, 512], f32, tag="s", name="k_ps",
                                   bufs=4)
                    for t in range(2):
                        nc.tensor.matmul(
                            k_ps[:],
                            cR[:, _RK + 256 * t + 128 * e:
                               _RK + 256 * t + 128 * e + 128],
                            xt[t][f // 4][:, (f % 4) * 512:(f % 4 + 1) * 512],
                            start=(t == 0), stop=(t == 1))
                    nc.scalar.activation(kt[e][f][:], k_ps[:],
                                         AF.Identity)

            def vvproj(u):
                v_ps = ps.tile([128, 512], f32, tag="s", name="v_ps", bufs=4)
                for jj in range(2):
                    jt = 2 * u + jj
                    for t in range(2):
                        nc.tensor.matmul(
                            v_ps[:, jj * 256:jj * 256 + 256],
                            xt[t][jt // 16][:, (jt % 16) * 128:
                                            (jt % 16) * 128 + 128],
                            wovT[t], start=(t == 0), stop=(t == 1))
                nc.scalar.activation(vvp[u][:], v_ps[:], AF.Identity)

            kproj(0)
            kproj(1)
            qproj(0)
            qproj(1)
            vvproj(0)

            # ---- attention: 2 supers x 2 chunks x 32 key tiles ----
            lacc = [resp.tile([128, IC], f32r, tag=f"lacc{c}", name=f"lacc{c}",
                              bufs=1) for c in range(NIC)]
            osb = [[None, None] for c in range(NIC)]
            rx = [[None, None] for c in range(NIC)]
            plast = {}
            rbt = {}

            def make_rx(c):
                for m in range(2):
                    r = resp.tile([128, IC], f32, tag=f"rx{c}_{m}",
                                  name=f"rx{c}_{m}", bufs=1)
                    nc.vector.tensor_scalar_add(
                        r[:], xc[m][0][:, c * IC:(c + 1) * IC], bfin[m])
                    rx[c][m] = r

            def denom(c):
                # softmax denominator for chunk c -> reciprocal rbt[c]
                lbc_ps = ps.tile([128, IC], f32, tag="s", name="lbc_ps",
                                 bufs=4)
                nc.tensor.matmul(lbc_ps[:], ones128, lacc[c][:],
                                 start=True, stop=False)
                p30, p31 = plast[c]
                nc.tensor.matmul(lbc_ps[:], ones_h, p30[:],
                                 start=False, stop=False)
                nc.tensor.matmul(lbc_ps[:], ones_h, p31[:],
                                 start=False, stop=True)
                rb = resp.tile([128, IC], f32, tag=f"rb{c}", name=f"rb{c}",
                               bufs=1)
                nc.vector.reciprocal_approx_fast(rb[:], lbc_ps[:])
                rbt[c] = rb

            def finalize0(sc):
                # deferred finalize for super-0 chunks (hidden mid-stream):
                # GpSimd adds, merged [128,1024] out DMA per channel-half
                denom(2 * sc)
                denom(2 * sc + 1)
                for m in range(2):
                    res = resp.tile([128, 1024], f32, tag=f"res0_{m}",
                                    name=f"res0_{m}", bufs=1)
                    for ci in range(2):
                        c = 2 * sc + ci
                        scaled = resp.tile([128, IC], f32, tag="scaled",
                                           name="scaled")
                        nc.vector.tensor_mul(scaled[:], osb[c][m][:],
                                             rbt[c][:])
                        nc.gpsimd.tensor_add(res[:, ci * IC:(ci + 1) * IC],
                                             scaled[:], rx[c][m][:])
                    nc.sync.dma_start(out_e[m, :, 2 * sc * IC:
                                            (2 * sc + 2) * IC], res[:])

            for sc in range(2):
                ca, cb = 2 * sc, 2 * sc + 1
                pv_ps = [[ps.tile([128, IC], f32, tag=f"pv{ci}_{m}",
                                  name=f"pv{ci}_{m}", bufs=1)
                          for m in range(2)] for ci in range(2)]

                def scores_block(jt):
                    s_a = ps.tile([128, IC], f32, tag="s", name="s_a", bufs=4)
                    s_b = ps.tile([128, IC], f32, tag="s", name="s_b", bufs=4)
                    for e in range(2):
                        ktile = kt[e][jt // 4][:, (jt % 4) * 128:(jt % 4 + 1) * 128]
                        nc.tensor.matmul(s_a[:], ktile, qt[e][ca][:],
                                         start=(e == 0), stop=(e == 1))
                        nc.tensor.matmul(s_b[:], ktile, qt[e][cb][:],
                                         start=(e == 0), stop=(e == 1))
                    ptag = "pt" if jt < NJT - 2 else "pfin"
                    pbufs = {} if jt < NJT - 2 else {"bufs": 4}
                    p_a = ptp.tile([128, IC], bf16, tag=ptag, name=ptag,
                                   **pbufs)
                    nc.scalar.activation(p_a[:], s_a[:], AF.Exp,
                                         bias=kconst[:, 0:1])
                    p_b = ptp.tile([128, IC], bf16, tag=ptag, name=ptag,
                                   **pbufs)
                    nc.scalar.activation(p_b[:], s_b[:], AF.Exp,
                                         bias=kconst[:, 0:1])
                    if jt == NJT - 2:
                        plast[ca] = [p_a, None]
                        plast[cb] = [p_b, None]
                    elif jt == NJT - 1:
                        plast[ca][1] = p_a
                        plast[cb][1] = p_b
                    return p_a, p_b

                # software pipeline: scores/exp emitted one jt ahead of PV
                # so the PE queue never stalls waiting on the exp latency
                p_next = scores_block(0)
                for jt in range(NJT):
                    p_a, p_b = p_next
                    if jt + 1 < NJT:
                        p_next = scores_block(jt + 1)
                    if sc == 1 and jt == NJT - 1:
                        # hoist the denominators + reciprocals ahead of the
                        # final PV matmuls so only mul/add/DMA trail the PE
                        denom(ca)
                        denom(cb)
                    for m in range(2):
                        vslice = vvp[jt // 2][:, (jt % 2) * 256 + m * 128:
                                              (jt % 2) * 256 + m * 128 + 128]
                        nc.tensor.matmul(pv_ps[0][m][:], vslice, p_a[:],
                                         start=(jt == 0), stop=(jt == NJT - 1))
                        nc.tensor.matmul(pv_ps[1][m][:], vslice, p_b[:],
                                         start=(jt == 0), stop=(jt == NJT - 1))
                    if sc == 0:
                        if jt % 4 == 0 and 4 <= jt <= 24:
                            kproj(jt // 4 + 1)
                        if jt == 2:
                            qproj(2)
                        if jt == 6:
                            qproj(3)
                        if jt % 2 == 0 and jt < NJT - 2:
                            vvproj(jt // 2 + 1)
                    if jt == 0:
                        nc.vector.tensor_copy(lacc[ca][:], p_a[:])
                        nc.gpsimd.tensor_copy(lacc[cb][:], p_b[:])
                    elif jt < NJT - 2:
                        nc.vector.tensor_add(lacc[ca][:], lacc[ca][:], p_a[:])
                        nc.gpsimd.tensor_add(lacc[cb][:], lacc[cb][:], p_b[:])
                    if sc == 1 and jt == 6:
                        finalize0(0)
                    if sc == 1 and jt == 20:
                        make_rx(2)
                        make_rx(3)
                if sc == 0:
                    make_rx(0)
                    make_rx(1)
                    # evacuate pv PSUM so super1 can reuse the banks
                    for ci in range(2):
                        for m in range(2):
                            o = resp.tile([128, IC], f32r, tag=f"osb{ci}_{m}",
                                          name=f"osb{ci}_{m}", bufs=1)
                            nc.vector.tensor_copy(o[:], pv_ps[ci][m][:])
                            osb[2 * sc + ci][m] = o
                else:
                    # closing finalize: Vector does the PSUM-reading muls
                    # (GpSimd cannot access PSUM); m=0 adds on Vector, m=1
                    # adds on GpSimd; each add fires its own [128,512] DMA
                    # immediately so nothing waits on unrelated writers
                    scl = [[resp.tile([128, IC], f32, tag=f"scl{ci}_{m}",
                                      name=f"scl{ci}_{m}", bufs=1)
                            for m in range(2)] for ci in range(2)]
                    for ci in range(2):
                        for m in range(2):
                            nc.vector.tensor_mul(scl[ci][m][:],
                                                 pv_ps[ci][m][:],
                                                 rbt[2 * sc + ci][:])
                    resq = {}
                    for ci in range(2):
                        c = 2 * sc + ci
                        r0 = resp.tile([128, IC], f32, tag=f"rq{ci}_0",
                                       name=f"rq{ci}_0", bufs=1)
                        nc.vector.tensor_add(r0[:], scl[ci][0][:],
                                             rx[c][0][:])
                        resq[(ci, 0)] = r0
                        r1 = resp.tile([128, IC], f32, tag=f"rq{ci}_1",
                                       name=f"rq{ci}_1", bufs=1)
                        nc.gpsimd.tensor_add(r1[:], scl[ci][1][:],
                                             rx[c][1][:])
                        resq[(ci, 1)] = r1
                        for m in range(2):
                            nc.sync.dma_start(
                                out_e[m, :, c * IC:(c + 1) * IC],
                                resq[(ci, m)][:])

    nc.compile()
    return nc


def _prep_inputs(x, gn_gamma, gn_beta, wq, bq, wk, bk, wv, bv, wo, bo):
    f = np.float32
    constR = np.zeros((128, _RCOLS), np.float16)
    wov = (wo.astype(f) @ wv.astype(f))
    for base, w in ((_RQ, wq), (_RK, wk), (_RO, wov)):
        wT = w.astype(f).T  # [c_in, c_out]
        for t in range(2):
            constR[:, base + 256 * t:base + 256 * t + 256] = \
                wT[128 * t:128 * (t + 1), :]
    constF = np.zeros((128, _FCOLS), f)
    gsel = np.zeros((2, 128, GROUPS), f)
    gselT = np.zeros((2, GROUPS, 128), f)
    for t in range(2):
        for p in range(128):
            g = (t * 128 + p) // CPG
            gsel[t, p, g] = 1.0
            gselT[t, g, p] = 1.0
    for t in range(2):
        constF[:, _FGS + 32 * t:_FGS + 32 * t + 32] = gsel[t]
        constF[0:GROUPS, _FGT + 128 * t:_FGT + 128 * t + 128] = gselT[t]
    wobvbo = (wo.astype(f) @ bv.astype(f) + bo.astype(f))
    vecs = (gn_gamma, gn_beta, bq, wobvbo)
    for i, v in enumerate(vecs):
        vv = v.astype(f).reshape(2, 128)
        for t in range(2):
            constF[:, _FVEC + 2 * i + t] = vv[t]
    constF[:, _FKC + 0] = -SHIFT
    constF[:, _FKC + 1] = EPS

    common = dict(constR=constR, constF=constF)
    xb = x.reshape(B, C, N).astype(np.float16)
    in_maps = []
    for core in range(NCORES):
        bi, qh = core // 2, core % 2
        xcore = xb[bi]
        if qh:
            xcore = np.concatenate([xcore[:, NQ:], xcore[:, :NQ]], axis=1)
        in_maps.append(dict(x=np.ascontiguousarray(xcore.reshape(2, 128, N)),
                            **common))
    return in_maps


def _execute(inputs, trace=False, **kw):
    from concourse.bass_utils import run_bass_kernel_spmd
    if "nc" not in _cache:
        _cache["nc"] = _build()
    nc = _cache["nc"]
    in_maps = _prep_inputs(**inputs)
    res = run_bass_kernel_spmd(nc, in_maps, core_ids=list(range(NCORES)),
                               trace=trace, **kw)
    out = np.empty((B, C, N), np.float32)
    for core in range(NCORES):
        bi, qh = core // 2, core % 2
        chunk = res.results[core]["out"].reshape(C, NQ)
        out[bi, :, qh * NQ:(qh + 1) * NQ] = chunk
    return out.reshape(B, C, DD, HH, WW), res


def kernel(**inputs):
    out, _ = _execute(inputs, trace=False)
    return out


# revision 14
# speedup vs baseline: 1.0787x; 1.0058x over previous
"""AttnBlock3d on 8 TRN2 NeuronCores.

Sharding: 8 cores = 4 batches x 2 query-halves. Each core receives its
batch's full x (rotated so its query half is always voxels [0:2048] --
GroupNorm and the attention key-reduction are voxel-permutation
invariant, so all cores run an identical graph), computes GN stats +
QKV + full attention for its 2048 queries and writes a [2,128,2048]
channel-tiled chunk.

Math restructuring vs the reference (exact up to fp rounding):
- x is shipped as fp16 (halves the HBM load; fp16 keeps 11 mantissa
  bits so the scores stay accurate, unlike bf16).  The GN apply is
  folded into a per-channel scale of the MOVING x: xt = a*x with
  a = gamma*rstd; projections use the RAW f32r transposed weights as
  stationaries (mixed fp16 x bass-matmul).  The b-terms reduce to
  per-query score shifts (softmax-invariant, dropped) plus the q bias
  cq = Wq b + bq (tiny on-chip matmuls); k needs no bias at all.
- The output projection is fused into V: vv = (Wo Wv) xt (Wo@Wv
  precomputed on host) so PV directly produces the o-projected output;
  the remaining constant (WoWv b + Wo bv + bo) is added at the end.
- Scores are computed transposed (S^T = k^T q, f32r) so no transposes
  are needed; exp applies a constant shift (scores stay in [-97,97]).
  The softmax denominator accumulates P^T tiles on Vector/GpSimd,
  summed across partitions by a ones-matmul; normalization is applied
  at the very end (linearity).
- The x load is CHAINED (tiny guard-DMAs on the Sync queue serialize
  the piece triggers) so pieces arrive in order and the GN stats
  (Scalar: sum-accum; Vector: tensor_tensor_reduce sumsq) overlap the
  load instead of serializing after it.
- The attention loop processes TWO 512-query chunks per key-tile pass
  so each stationary tile serves 2 matmuls.
- The last super's softmax denominators + reciprocals are hoisted
  before the final PV matmuls; the closing normalize+residual is split
  Vector/GpSimd and the output DMA is merged to one [128,1024] write
  per channel-half, so almost nothing trails the last matmul.
- The DMA-load head warms the PE (HAM) with junk matmuls paced by x
  piece arrivals, and activation tables (Identity/Sqrt/Exp) are
  preloaded with dummies so no table load lands on the critical path.
"""

import sys

for _p in ("/opt/trn_rl_repo",):
    if _p not in sys.path:
        sys.path.append(_p)

import numpy as np

B, C, DD, HH, WW = 4, 256, 16, 16, 16
N = DD * HH * WW          # 4096 voxels
NQ = N // 2               # queries per core
GROUPS = 32
CPG = C // GROUPS         # channels per group
EPS = 1e-6
SHIFT = 60.0              # softmax constant shift
NCORES = 8
IC = 512                  # query chunk
NIC = NQ // IC            # 4 chunks
NJT = N // 128            # 32 key tiles
XC = 2048                 # x-load / GN piece (voxels)
NXC = N // XC             # 2 pieces per channel-half

# packed-constant column offsets (constR: f32r, constF: f32)
_RQ, _RK, _RO = 0, 512, 1024
_RCOLS = 1536
_FGS, _FGT, _FVEC, _FKC = 0, 64, 320, 328
_FCOLS = 330

CHAIN = 1  # serialize x piece DMAs via guard-DMAs on the Sync queue

_cache = {}


def _build():
    import concourse.bass as bass
    from concourse import bacc, mybir, tile

    f32 = mybir.dt.float32
    f32r = mybir.dt.float32r
    f16 = mybir.dt.float16
    bf16 = mybir.dt.bfloat16
    AF = mybir.ActivationFunctionType
    OP = mybir.AluOpType
    AX = mybir.AxisListType

    nc = bacc.Bacc("TRN2", target_bir_lowering=False, debug=False,
                   num_devices=NCORES)

    x_e = nc.dram_tensor("x", [2, 128, N], f16, kind="ExternalInput").ap()
    cR_e = nc.dram_tensor("constR", [128, _RCOLS], f16,
                          kind="ExternalInput").ap()
    cF_e = nc.dram_tensor("constF", [128, _FCOLS], f32,
                          kind="ExternalInput").ap()
    out_e = nc.dram_tensor("out", [2, 128, NQ], bf16,
                           kind="ExternalOutput").ap()

    with tile.TileContext(nc) as tc:
        with tc.tile_pool(name="big", bufs=1) as big, \
             tc.tile_pool(name="w", bufs=1) as wp, \
             tc.tile_pool(name="sm", bufs=2) as sm, \
             tc.tile_pool(name="pt", bufs=12) as ptp, \
             tc.tile_pool(name="res", bufs=2) as resp, \
             tc.tile_pool(name="psum", bufs=1, space="PSUM") as ps:

            # ---- on-chip constants (no DMA dependency) ----
            ones128 = wp.tile([128, 128], f32r, tag="ones", name="ones128")
            ones_h = wp.tile([128, 128], bf16, tag="onesh", name="ones_h")
            ones_f = wp.tile([128, 128], f32, tag="onesf", name="ones_f")
            ones16 = wp.tile([128, 128], f16, tag="ones16", name="ones16")
            nc.vector.memset(ones_f[:], 1.0)
            nc.vector.tensor_copy(ones128[:], ones_f[:])
            nc.vector.tensor_copy(ones_h[:], ones_f[:])
            nc.vector.tensor_copy(ones16[:], ones_f[:])
            dzi = sm.tile([GROUPS, 1], f32, tag="dzi", name="dzi")
            dzo = sm.tile([GROUPS, 1], f32, tag="dzo", name="dzo")
            nc.vector.memset(dzi[:], 0.25)
            # preload the activation tables off the critical path
            nc.scalar.activation(dzo[:], dzi[:], AF.Identity)
            nc.scalar.activation(dzo[:], dzi[:], AF.Sqrt)
            nc.scalar.activation(dzo[:], dzi[:], AF.Exp)

            # ---- chained x load: pieces arrive IN ORDER so GN stats
            # overlap the load.  Tiny guard-DMAs on the Sync queue make the
            # engine wait for a piece before triggering later pieces. ----
            xc = [[big.tile([128, XC], f16, tag=f"x{t}_{cx}", name=f"x{t}_{cx}")
                   for cx in range(NXC)] for t in range(2)]
            cR = wp.tile([128, _RCOLS], f16, tag="cR", name="cR")
            cF = wp.tile([128, _FCOLS], f32, tag="cF", name="cF")
            gdst = sm.tile([1, 16], f16, tag="gdst", name="gdst")

            def trig(t, cx):
                sl = slice(cx * XC, (cx + 1) * XC)
                nc.sync.dma_start(xc[t][cx][:], x_e[t, :, sl])

            def guard(t, cx):
                nc.sync.dma_start(gdst[:], xc[t][cx][0:1, 0:16])

            # piece order: (0,0) (1,0) (0,1) (1,1)
            if CHAIN:
                trig(0, 0)
                guard(0, 0)
                trig(1, 0)
                nc.sync.dma_start(cF[:], cF_e[:])
                guard(1, 0)
                trig(0, 1)
                trig(1, 1)
                guard(0, 1)
                nc.sync.dma_start(cR[:], cR_e[:])
            else:
                trig(0, 0)
                trig(1, 0)
                nc.sync.dma_start(cF[:], cF_e[:])
                trig(0, 1)
                trig(1, 1)
                nc.sync.dma_start(cR[:], cR_e[:])

            # raw transposed weights [c_in, c_out]; blocks (2t+m)
            wqT = [[cR[:, _RQ + 128 * (2 * t + m):_RQ + 128 * (2 * t + m) + 128]
                    for m in range(2)] for t in range(2)]
            wovT = [cR[:, _RO + 256 * t:_RO + 256 * t + 256] for t in range(2)]
            gsel = [cF[:, _FGS + 32 * t:_FGS + 32 * t + 32] for t in range(2)]
            gselT = [cF[0:GROUPS, _FGT + 128 * t:_FGT + 128 * t + 128]
                     for t in range(2)]
            gamma = [cF[:, _FVEC + 0 + t:_FVEC + 1 + t] for t in range(2)]
            beta = [cF[:, _FVEC + 2 + t:_FVEC + 3 + t] for t in range(2)]
            bq = [cF[:, _FVEC + 4 + t:_FVEC + 5 + t] for t in range(2)]
            wobvbo = [cF[:, _FVEC + 6 + t:_FVEC + 7 + t] for t in range(2)]
            kconst = cF[:, _FKC:_FKC + 2]

            # ---- GN stats overlap the chained load: per piece Scalar does
            # the sum (Identity + accum into a junk out) and Vector does the
            # sumsq (tensor_tensor_reduce); PE warms (HAM) pace on pieces ----
            sum4 = [sm.tile([128, NXC], f32, tag=f"sum4{t}", name=f"sum4{t}")
                    for t in range(2)]
            sq4 = [sm.tile([128, NXC], f32, tag=f"sq4{t}", name=f"sq4{t}")
                   for t in range(2)]
            stats = [sm.tile([128, 2], f32, tag=f"st{t}", name=f"st{t}")
                     for t in range(2)]
            for cx in range(NXC):
                for t in range(2):
                    sj = sm.tile([128, XC], f16, tag="sjunk", name="sjunk",
                                 bufs=1)
                    nc.scalar.activation(sj[:], xc[t][cx][:], AF.Identity,
                                         accum_out=sum4[t][:, cx:cx + 1])
                    vj = sm.tile([128, XC], f16, tag="vjunk", name="vjunk",
                                 bufs=1)
                    nc.vector.tensor_mul(vj[:], xc[t][cx][:], xc[t][cx][:])
                    nc.vector.reduce_sum(sq4[t][:, cx:cx + 1], vj[:],
                                         axis=AX.X)
                    for r in range(3):
                        w_ps = ps.tile([128, IC], f32, tag="s", name="warm",
                                       bufs=4)
                        nc.tensor.matmul(w_ps[:], ones16,
                                         xc[t][cx][:, (r % 4) * 512:
                                                    (r % 4) * 512 + 512],
                                         start=True, stop=True)
            # dense warm burst on the last piece to flip HAM before the
            # real stream starts
            for r in range(10):
                t, co = r % 2, (r % 4) * 512
                w_ps = ps.tile([128, IC], f32, tag="s", name="warm", bufs=4)
                nc.tensor.matmul(w_ps[:], ones16,
                                 xc[t][NXC - 1][:, co:co + IC],
                                 start=True, stop=True)

            def warmmm(n):
                for r in range(n):
                    t, co = r % 2, (r % 4) * 512
                    w_ps = ps.tile([128, IC], f32, tag="s", name="warm",
                                   bufs=4)
                    nc.tensor.matmul(w_ps[:], ones16,
                                     xc[t][0][:, co:co + IC],
                                     start=True, stop=True)

            # ---- stats combine -> per-channel a, b ----
            for t in range(2):
                nc.vector.reduce_sum(stats[t][:, 0:1], sum4[t][:], axis=AX.X)
                nc.vector.reduce_sum(stats[t][:, 1:2], sq4[t][:], axis=AX.X)
            warmmm(3)
            g_ps = ps.tile([GROUPS, 2], f32, tag="s", name="g_ps", bufs=4)
            for t in range(2):
                nc.tensor.matmul(g_ps[:], gsel[t], stats[t][:],
                                 start=(t == 0), stop=(t == 1))
            for t in range(2):
                f_ps = ps.tile([128, 2], f32, tag="s", name="fill", bufs=4)
                nc.tensor.matmul(f_ps[:], ones_f, stats[t][:],
                                 start=True, stop=True)
                warmmm(2)
            gstats = sm.tile([GROUPS, 2], f32, tag="gstats", name="gstats")
            var = sm.tile([GROUPS, 1], f32, tag="gvar", name="gvar")
            stdt = sm.tile([GROUPS, 1], f32, tag="gstd", name="gstd")
            inv = 1.0 / (CPG * N)
            nc.vector.tensor_scalar_mul(gstats[:, 0:2], g_ps[:, 0:2], inv)
            # negvar = mean*mean - ex2 ; std = sqrt(-negvar + eps)
            nc.vector.scalar_tensor_tensor(var[:], gstats[:, 0:1],
                                           gstats[:, 0:1], gstats[:, 1:2],
                                           op0=OP.mult, op1=OP.subtract)
            nc.scalar.activation(stdt[:], var[:], AF.Sqrt,
                                 bias=kconst[0:GROUPS, 1:2], scale=-1.0)
            nc.vector.reciprocal_approx_fast(gstats[:, 1:2], stdt[:])
            ab = [sm.tile([128, 2], f32, tag=f"ab{t}", name=f"ab{t}")
                  for t in range(2)]
            bcol = [sm.tile([128, 2], f16, tag=f"bcol{t}", name=f"bcol{t}")
                    for t in range(2)]
            for t in range(2):
                bc_ps = ps.tile([128, 2], f32, tag="s", name="bc_ps", bufs=4)
                nc.tensor.matmul(bc_ps[:], gselT[t], gstats[:],
                                 start=True, stop=True)
                # a = rstd*gamma ; b = beta - mean*a
                nc.vector.tensor_mul(ab[t][:, 0:1], bc_ps[:, 1:2], gamma[t])
                nc.vector.tensor_mul(ab[t][:, 1:2], bc_ps[:, 0:1], ab[t][:, 0:1])
                nc.vector.tensor_sub(ab[t][:, 1:2], beta[t], ab[t][:, 1:2])
                nc.vector.tensor_copy(bcol[t][:, 0:1], ab[t][:, 1:2])
                nc.vector.tensor_copy(bcol[t][:, 1:2], ab[t][:, 1:2])

            f_ps = ps.tile([128, 2], f32, tag="s", name="fill", bufs=4)
            nc.tensor.matmul(f_ps[:], ones_f[0:GROUPS, :], gstats[:],
                             start=True, stop=True)
            warmmm(3)

            # ---- scaled moving x: xt = a * x (fp16), chunk 0 first ----
            xt = [[big.tile([128, XC], f16, tag=f"xt{t}_{cx}",
                            name=f"xt{t}_{cx}") for cx in range(NXC)]
                  for t in range(2)]
            for cx in range(NXC):
                for t in range(2):
                    nc.vector.tensor_scalar_mul(xt[t][cx][:], xc[t][cx][:],
                                                ab[t][:, 0:1])
                warmmm(2)

            # ---- on-chip bias columns: cq = Wq b + bq (per attention-channel
            # tile), bfin = WoWv b + (Wo bv + bo) ----
            cqc = [sm.tile([128, 1], f32, tag=f"cq{m}", name=f"cq{m}")
                   for m in range(2)]
            bfin = [sm.tile([128, 1], f32, tag=f"bf{m}", name=f"bf{m}")
                    for m in range(2)]
            for m in range(2):
                c_ps = ps.tile([128, 2], f32, tag="s", name="c_ps", bufs=4)
                for t in range(2):
                    nc.tensor.matmul(c_ps[:], wqT[t][m], bcol[t][:],
                                     start=(t == 0), stop=(t == 1))
                nc.vector.tensor_scalar_add(cqc[m][:], c_ps[:, 0:1], bq[m])
            for m in range(2):
                c_ps = ps.tile([128, 2], f32, tag="s", name="c_ps", bufs=4)
                for t in range(2):
                    nc.tensor.matmul(c_ps[:], wovT[t][:, m * 128:m * 128 + 128],
                                     bcol[t][:], start=(t == 0), stop=(t == 1))
                nc.vector.tensor_scalar_add(bfin[m][:], c_ps[:, 0:1], wobvbo[m])
            warmmm(2)

            # ---- projections on scaled x ----
            qt = [[big.tile([128, IC], f32r, tag=f"q{e}_{f}", name=f"q{e}_{f}")
                   for f in range(NIC)] for e in range(2)]
            kt = [[big.tile([128, 512], f32r, tag=f"k{e}_{f}", name=f"k{e}_{f}")
                   for f in range(N // 512)] for e in range(2)]
            # vv pairs: [vv(2u) | vv(2u+1)], each [128 keys, 256 ch] bf16
            vvp = [big.tile([128, 512], bf16, tag=f"vv{u}", name=f"vv{u}")
                   for u in range(NJT // 2)]

            def qproj(f):
                for e in range(2):
                    q_ps = ps.tile([128, IC], f32, tag="s", name="q_ps",
                                   bufs=4)
                    for t in range(2):
                        nc.tensor.matmul(
                            q_ps[:],
                            cR[:, _RQ + 256 * t + 128 * e:
                               _RQ + 256 * t + 128 * e + 128],
                            xt[t][0][:, f * 512:(f + 1) * 512],
                            start=(t == 0), stop=(t == 1))
                    nc.vector.tensor_scalar_add(qt[e][f][:], q_ps[:], cqc[e])

            def kproj(f):
                for e in range(2):
                    k_ps = ps.tile([128, 512], f32, tag="s", name="k_ps",
                                   bufs=4)
                    for t in range(2):
                        nc.tensor.matmul(
                            k_ps[:],
                            cR[:, _RK + 256 * t + 128 * e:
                               _RK + 256 * t + 128 * e + 128],
                            xt[t][f // 4][:, (f % 4) * 512:(f % 4 + 1) * 512],
                            start=(t == 0), stop=(t == 1))
                    nc.scalar.activation(kt[e][f][:], k_ps[:],
                                         AF.Identity)

            def vvproj(u):
                v_ps = ps.tile([128, 512], f32, tag="s", name="v_ps", bufs=4)
                for jj in range(2):
                    jt = 2 * u + jj
                    for t in range(2):
                        nc.tensor.matmul(
                            v_ps[:, jj * 256:jj * 256 + 256],
                            xt[t][jt // 16][:, (jt % 16) * 128:
                                            (jt % 16) * 128 + 128],
                            wovT[t], start=(t == 0), stop=(t == 1))
                nc.scalar.activation(vvp[u][:], v_ps[:], AF.Identity)

            kproj(0)
            kproj(1)
            qproj(0)
            qproj(1)
            vvproj(0)

            # ---- attention: 2 supers x 2 chunks x 32 key tiles ----
            lacc = [resp.tile([128, IC], f32r, tag=f"lacc{c}", name=f"lacc{c}",
                              bufs=1) for c in range(NIC)]
            osb = [[None, None] for c in range(NIC)]
            rx = [[None, None] for c in range(NIC)]
            plast = {}
            rbt = {}

            def make_rx(c):
                for m in range(2):
                    r = resp.tile([128, IC], f32, tag=f"rx{c}_{m}",
                                  name=f"rx{c}_{m}", bufs=1)
                    nc.vector.tensor_scalar_add(
                        r[:], xc[m][0][:, c * IC:(c + 1) * IC], bfin[m])
                    rx[c][m] = r

            def denom(c):
                # softmax denominator for chunk c -> reciprocal rbt[c]
                lbc_ps = ps.tile([128, IC], f32, tag="s", name="lbc_ps",
                                 bufs=4)
                nc.tensor.matmul(lbc_ps[:], ones128, lacc[c][:],
                                 start=True, stop=False)
                p30, p31 = plast[c]
                nc.tensor.matmul(lbc_ps[:], ones_h, p30[:],
                                 start=False, stop=False)
                nc.tensor.matmul(lbc_ps[:], ones_h, p31[:],
                                 start=False, stop=True)
                rb = resp.tile([128, IC], f32, tag=f"rb{c}", name=f"rb{c}",
                               bufs=1)
                nc.vector.reciprocal_approx_fast(rb[:], lbc_ps[:])
                rbt[c] = rb

            def finalize0(sc):
                # deferred finalize for super-0 chunks (hidden mid-stream):
                # GpSimd adds, merged [128,1024] out DMA per channel-half
                denom(2 * sc)
                denom(2 * sc + 1)
                for m in range(2):
                    res = resp.tile([128, 1024], bf16, tag=f"res0_{m}",
                                    name=f"res0_{m}", bufs=1)
                    for ci in range(2):
                        c = 2 * sc + ci
                        scaled = resp.tile([128, IC], f32, tag="scaled",
                                           name="scaled")
                        nc.vector.tensor_mul(scaled[:], osb[c][m][:],
                                             rbt[c][:])
                        nc.gpsimd.tensor_add(res[:, ci * IC:(ci + 1) * IC],
                                             scaled[:], rx[c][m][:])
                    nc.sync.dma_start(out_e[m, :, 2 * sc * IC:
                                            (2 * sc + 2) * IC], res[:])

            for sc in range(2):
                ca, cb = 2 * sc, 2 * sc + 1
                pv_ps = [[ps.tile([128, IC], f32, tag=f"pv{ci}_{m}",
                                  name=f"pv{ci}_{m}", bufs=1)
                          for m in range(2)] for ci in range(2)]

                def scores_block(jt):
                    s_a = ps.tile([128, IC], f32, tag="s", name="s_a", bufs=4)
                    s_b = ps.tile([128, IC], f32, tag="s", name="s_b", bufs=4)
                    for e in range(2):
                        ktile = kt[e][jt // 4][:, (jt % 4) * 128:(jt % 4 + 1) * 128]
                        nc.tensor.matmul(s_a[:], ktile, qt[e][ca][:],
                                         start=(e == 0), stop=(e == 1))
                        nc.tensor.matmul(s_b[:], ktile, qt[e][cb][:],
                                         start=(e == 0), stop=(e == 1))
                    ptag = "pt" if jt < NJT - 2 else "pfin"
                    pbufs = {} if jt < NJT - 2 else {"bufs": 4}
                    p_a = ptp.tile([128, IC], bf16, tag=ptag, name=ptag,
                                   **pbufs)
                    nc.scalar.activation(p_a[:], s_a[:], AF.Exp,
                                         bias=kconst[:, 0:1])
                    p_b = ptp.tile([128, IC], bf16, tag=ptag, name=ptag,
                                   **pbufs)
                    nc.scalar.activation(p_b[:], s_b[:], AF.Exp,
                                         bias=kconst[:, 0:1])
                    if jt == NJT - 2:
                        plast[ca] = [p_a, None]
                        plast[cb] = [p_b, None]
                    elif jt == NJT - 1:
                        plast[ca][1] = p_a
                        plast[cb][1] = p_b
                    return p_a, p_b

                # software pipeline: scores/exp emitted one jt ahead of PV
                # so the PE queue never stalls waiting on the exp latency
                p_next = scores_block(0)
                for jt in range(NJT):
                    p_a, p_b = p_next
                    if jt + 1 < NJT:
                        p_next = scores_block(jt + 1)
                    if sc == 1 and jt == NJT - 1:
                        # hoist the denominators + reciprocals ahead of the
                        # final PV matmuls so only mul/add/DMA trail the PE
                        denom(ca)
                        denom(cb)
                    for m in range(2):
                        vslice = vvp[jt // 2][:, (jt % 2) * 256 + m * 128:
                                              (jt % 2) * 256 + m * 128 + 128]
                        nc.tensor.matmul(pv_ps[0][m][:], vslice, p_a[:],
                                         start=(jt == 0), stop=(jt == NJT - 1))
                        nc.tensor.matmul(pv_ps[1][m][:], vslice, p_b[:],
                                         start=(jt == 0), stop=(jt == NJT - 1))
                    if sc == 0:
                        if jt % 4 == 0 and 4 <= jt <= 24:
                            kproj(jt // 4 + 1)
                        if jt == 2:
                            qproj(2)
                        if jt == 6:
                            qproj(3)
                        if jt % 2 == 0 and jt < NJT - 2:
                            vvproj(jt // 2 + 1)
                    if jt == 0:
                        nc.vector.tensor_copy(lacc[ca][:], p_a[:])
                        nc.gpsimd.tensor_copy(lacc[cb][:], p_b[:])
                    elif jt < NJT - 2:
                        nc.vector.tensor_add(lacc[ca][:], lacc[ca][:], p_a[:])
                        nc.gpsimd.tensor_add(lacc[cb][:], lacc[cb][:], p_b[:])
                    if sc == 1 and jt == 6:
                        finalize0(0)
                    if sc == 1 and jt == 20:
                        make_rx(2)
                        make_rx(3)
                if sc == 0:
                    make_rx(0)
                    make_rx(1)
                    # evacuate pv PSUM so super1 can reuse the banks
                    for ci in range(2):
                        for m in range(2):
                            o = resp.tile([128, IC], f32r, tag=f"osb{ci}_{m}",
                                          name=f"osb{ci}_{m}", bufs=1)
                            nc.vector.tensor_copy(o[:], pv_ps[ci][m][:])
                            osb[2 * sc + ci][m] = o
                else:
                    # closing finalize: Vector does the PSUM-reading muls
                    # (GpSimd cannot access PSUM) + m=0 adds; GpSimd does the
                    # m=1 adds; bf16 res halves the exposed out-DMA bytes and
                    # each m's merged [128,1024] DMA fires right after its
                    # own adds
                    scl = [[resp.tile([128, IC], f32, tag=f"scl{ci}_{m}",
                                      name=f"scl{ci}_{m}", bufs=1)
                            for m in range(2)] for ci in range(2)]
                    for ci in range(2):
                        for m in range(2):
                            nc.vector.tensor_mul(scl[ci][m][:],
                                                 pv_ps[ci][m][:],
                                                 rbt[2 * sc + ci][:])
                    res1 = [resp.tile([128, 1024], bf16, tag=f"res1_{m}",
                                      name=f"res1_{m}", bufs=1)
                            for m in range(2)]
                    for ci in range(2):
                        nc.vector.tensor_add(
                            res1[0][:, ci * IC:(ci + 1) * IC],
                            scl[ci][0][:], rx[2 * sc + ci][0][:])
                    nc.sync.dma_start(out_e[0, :, 2 * sc * IC:
                                            (2 * sc + 2) * IC], res1[0][:])
                    for ci in range(2):
                        nc.gpsimd.tensor_add(
                            res1[1][:, ci * IC:(ci + 1) * IC],
                            scl[ci][1][:], rx[2 * sc + ci][1][:])
                    nc.sync.dma_start(out_e[1, :, 2 * sc * IC:
                                            (2 * sc + 2) * IC], res1[1][:])

    nc.compile()
    return nc


def _prep_inputs(x, gn_gamma, gn_beta, wq, bq, wk, bk, wv, bv, wo, bo):
    f = np.float32
    constR = np.zeros((128, _RCOLS), np.float16)
    wov = (wo.astype(f) @ wv.astype(f))
    for base, w in ((_RQ, wq), (_RK, wk), (_RO, wov)):
        wT = w.astype(f).T  # [c_in, c_out]
        for t in range(2):
            constR[:, base + 256 * t:base + 256 * t + 256] = \
                wT[128 * t:128 * (t + 1), :]
    constF = np.zeros((128, _FCOLS), f)
    gsel = np.zeros((2, 128, GROUPS), f)
    gselT = np.zeros((2, GROUPS, 128), f)
    for t in range(2):
        for p in range(128):
            g = (t * 128 + p) // CPG
            gsel[t, p, g] = 1.0
            gselT[t, g, p] = 1.0
    for t in range(2):
        constF[:, _FGS + 32 * t:_FGS + 32 * t + 32] = gsel[t]
        constF[0:GROUPS, _FGT + 128 * t:_FGT + 128 * t + 128] = gselT[t]
    wobvbo = (wo.astype(f) @ bv.astype(f) + bo.astype(f))
    vecs = (gn_gamma, gn_beta, bq, wobvbo)
    for i, v in enumerate(vecs):
        vv = v.astype(f).reshape(2, 128)
        for t in range(2):
            constF[:, _FVEC + 2 * i + t] = vv[t]
    constF[:, _FKC + 0] = -SHIFT
    constF[:, _FKC + 1] = EPS

    common = dict(constR=constR, constF=constF)
    xb = x.reshape(B, C, N).astype(np.float16)
    in_maps = []
    for core in range(NCORES):
        bi, qh = core // 2, core % 2
        xcore = xb[bi]
        if qh:
            xcore = np.concatenate([xcore[:, NQ:], xcore[:, :NQ]], axis=1)
        in_maps.append(dict(x=np.ascontiguousarray(xcore.reshape(2, 128, N)),
                            **common))
    return in_maps


def _execute(inputs, trace=False, **kw):
    from concourse.bass_utils import run_bass_kernel_spmd
    if "nc" not in _cache:
        _cache["nc"] = _build()
    nc = _cache["nc"]
    in_maps = _prep_inputs(**inputs)
    res = run_bass_kernel_spmd(nc, in_maps, core_ids=list(range(NCORES)),
                               trace=trace, **kw)
    out = np.empty((B, C, N), np.float32)
    for core in range(NCORES):
        bi, qh = core // 2, core % 2
        chunk = np.asarray(res.results[core]["out"]).astype(
            np.float32).reshape(C, NQ)
        out[bi, :, qh * NQ:(qh + 1) * NQ] = chunk
    return out.reshape(B, C, DD, HH, WW), res


def kernel(**inputs):
    out, _ = _execute(inputs, trace=False)
    return out


# revision 15
# speedup vs baseline: 1.0858x; 1.0066x over previous
"""AttnBlock3d on 8 TRN2 NeuronCores.

Sharding: 8 cores = 4 batches x 2 query-halves. Each core receives its
batch's full x (rotated so its query half is always voxels [0:2048] --
GroupNorm and the attention key-reduction are voxel-permutation
invariant, so all cores run an identical graph), computes GN stats +
QKV + full attention for its 2048 queries and writes a [2,128,2048]
channel-tiled chunk.

Math restructuring vs the reference (exact up to fp rounding):
- x is shipped as fp16 (halves the HBM load; fp16 keeps 11 mantissa
  bits so the scores stay accurate, unlike bf16).  The GN apply is
  folded into a per-channel scale of the MOVING x: xt = a*x with
  a = gamma*rstd; projections use the RAW f32r transposed weights as
  stationaries (mixed fp16 x bass-matmul).  The b-terms reduce to
  per-query score shifts (softmax-invariant, dropped) plus the q bias
  cq = Wq b + bq (tiny on-chip matmuls); k needs no bias at all.
- The output projection is fused into V: vv = (Wo Wv) xt (Wo@Wv
  precomputed on host) so PV directly produces the o-projected output;
  the remaining constant (WoWv b + Wo bv + bo) is added at the end.
- Scores are computed transposed (S^T = k^T q, f32r) so no transposes
  are needed; exp applies a constant shift (scores stay in [-97,97]).
  The softmax denominator accumulates P^T tiles on Vector/GpSimd,
  summed across partitions by a ones-matmul; normalization is applied
  at the very end (linearity).
- The x load is CHAINED (tiny guard-DMAs on the Sync queue serialize
  the piece triggers) so pieces arrive in order and the GN stats
  (Scalar: sum-accum; Vector: tensor_tensor_reduce sumsq) overlap the
  load instead of serializing after it.
- The attention loop processes TWO 512-query chunks per key-tile pass
  so each stationary tile serves 2 matmuls.
- The last super's softmax denominators + reciprocals are hoisted
  before the final PV matmuls; the closing normalize+residual is split
  Vector/GpSimd and the output DMA is merged to one [128,1024] write
  per channel-half, so almost nothing trails the last matmul.
- The DMA-load head warms the PE (HAM) with junk matmuls paced by x
  piece arrivals, and activation tables (Identity/Sqrt/Exp) are
  preloaded with dummies so no table load lands on the critical path.
"""

import sys

for _p in ("/opt/trn_rl_repo",):
    if _p not in sys.path:
        sys.path.append(_p)

import numpy as np

B, C, DD, HH, WW = 4, 256, 16, 16, 16
N = DD * HH * WW          # 4096 voxels
NQ = N // 2               # queries per core
GROUPS = 32
CPG = C // GROUPS         # channels per group
EPS = 1e-6
SHIFT = 60.0              # softmax constant shift
NCORES = 8
IC = 512                  # query chunk
NIC = NQ // IC            # 4 chunks
NJT = N // 128            # 32 key tiles
XC = 2048                 # x-load / GN piece (voxels)
NXC = N // XC             # 2 pieces per channel-half

# packed-constant column offsets (constR: f32r, constF: f32)
_RQ, _RK, _RO = 0, 512, 1024
_RCOLS = 1536
_FGS, _FGT, _FVEC, _FKC = 0, 64, 320, 328
_FCOLS = 330

CHAIN = 1  # serialize x piece DMAs via guard-DMAs on the Sync queue

_cache = {}


def _build():
    import concourse.bass as bass
    from concourse import bacc, mybir, tile

    f32 = mybir.dt.float32
    f32r = mybir.dt.float32r
    f16 = mybir.dt.float16
    bf16 = mybir.dt.bfloat16
    AF = mybir.ActivationFunctionType
    OP = mybir.AluOpType
    AX = mybir.AxisListType

    nc = bacc.Bacc("TRN2", target_bir_lowering=False, debug=False,
                   num_devices=NCORES)

    x_e = nc.dram_tensor("x", [2, 128, N], f16, kind="ExternalInput").ap()
    cR_e = nc.dram_tensor("constR", [128, _RCOLS], f16,
                          kind="ExternalInput").ap()
    cF_e = nc.dram_tensor("constF", [128, _FCOLS], f32,
                          kind="ExternalInput").ap()
    out_e = nc.dram_tensor("out", [2, 128, NQ], bf16,
                           kind="ExternalOutput").ap()

    with tile.TileContext(nc) as tc:
        with tc.tile_pool(name="big", bufs=1) as big, \
             tc.tile_pool(name="w", bufs=1) as wp, \
             tc.tile_pool(name="sm", bufs=2) as sm, \
             tc.tile_pool(name="pt", bufs=12) as ptp, \
             tc.tile_pool(name="res", bufs=2) as resp, \
             tc.tile_pool(name="psum", bufs=1, space="PSUM") as ps:

            # ---- on-chip constants (no DMA dependency) ----
            ones128 = wp.tile([128, 128], f32r, tag="ones", name="ones128")
            ones_h = wp.tile([128, 128], bf16, tag="onesh", name="ones_h")
            ones_f = wp.tile([128, 128], f32, tag="onesf", name="ones_f")
            ones16 = wp.tile([128, 128], f16, tag="ones16", name="ones16")
            nc.vector.memset(ones_f[:], 1.0)
            nc.vector.tensor_copy(ones128[:], ones_f[:])
            nc.vector.tensor_copy(ones_h[:], ones_f[:])
            nc.vector.tensor_copy(ones16[:], ones_f[:])
            dzi = sm.tile([GROUPS, 1], f32, tag="dzi", name="dzi")
            dzo = sm.tile([GROUPS, 1], f32, tag="dzo", name="dzo")
            nc.vector.memset(dzi[:], 0.25)
            # preload the activation tables off the critical path
            nc.scalar.activation(dzo[:], dzi[:], AF.Identity)
            nc.scalar.activation(dzo[:], dzi[:], AF.Sqrt)
            nc.scalar.activation(dzo[:], dzi[:], AF.Exp)
            # pre-warm the engines (DVFS ramps after ~3us of activity): PE
            # warms on a constant junk tile until x arrives; Vector/GpSimd
            # run junk ops so the stats hit full-rate engines
            jk = wp.tile([128, 512], f16, tag="jk", name="jk")
            jk2 = wp.tile([128, 512], f16, tag="jk2", name="jk2")
            nc.vector.memset(jk[:], 1.0)
            for r in range(6):
                nc.vector.tensor_copy(jk2[:], jk[:])
            for r in range(2):
                nc.gpsimd.tensor_copy(jk2[:], jk[:])
            for r in range(10):
                w_ps = ps.tile([128, IC], f32, tag="s", name="warm", bufs=4)
                nc.tensor.matmul(w_ps[:], ones16, jk[:], start=True,
                                 stop=True)

            # ---- chained x load: pieces arrive IN ORDER so GN stats
            # overlap the load.  Tiny guard-DMAs on the Sync queue make the
            # engine wait for a piece before triggering later pieces. ----
            xc = [[big.tile([128, XC], f16, tag=f"x{t}_{cx}", name=f"x{t}_{cx}")
                   for cx in range(NXC)] for t in range(2)]
            cR = wp.tile([128, _RCOLS], f16, tag="cR", name="cR")
            cF = wp.tile([128, _FCOLS], f32, tag="cF", name="cF")
            gdst = sm.tile([1, 16], f16, tag="gdst", name="gdst")

            def trig(t, cx):
                sl = slice(cx * XC, (cx + 1) * XC)
                nc.sync.dma_start(xc[t][cx][:], x_e[t, :, sl])

            def guard(t, cx):
                nc.sync.dma_start(gdst[:], xc[t][cx][0:1, 0:16])

            # piece order: (0,0) (1,0) (0,1) (1,1)
            if CHAIN:
                trig(0, 0)
                guard(0, 0)
                trig(1, 0)
                nc.sync.dma_start(cF[:], cF_e[:])
                guard(1, 0)
                trig(0, 1)
                trig(1, 1)
                guard(0, 1)
                nc.sync.dma_start(cR[:], cR_e[:])
            else:
                trig(0, 0)
                trig(1, 0)
                nc.sync.dma_start(cF[:], cF_e[:])
                trig(0, 1)
                trig(1, 1)
                nc.sync.dma_start(cR[:], cR_e[:])

            # raw transposed weights [c_in, c_out]; blocks (2t+m)
            wqT = [[cR[:, _RQ + 128 * (2 * t + m):_RQ + 128 * (2 * t + m) + 128]
                    for m in range(2)] for t in range(2)]
            wovT = [cR[:, _RO + 256 * t:_RO + 256 * t + 256] for t in range(2)]
            gsel = [cF[:, _FGS + 32 * t:_FGS + 32 * t + 32] for t in range(2)]
            gselT = [cF[0:GROUPS, _FGT + 128 * t:_FGT + 128 * t + 128]
                     for t in range(2)]
            gamma = [cF[:, _FVEC + 0 + t:_FVEC + 1 + t] for t in range(2)]
            beta = [cF[:, _FVEC + 2 + t:_FVEC + 3 + t] for t in range(2)]
            bq = [cF[:, _FVEC + 4 + t:_FVEC + 5 + t] for t in range(2)]
            wobvbo = [cF[:, _FVEC + 6 + t:_FVEC + 7 + t] for t in range(2)]
            kconst = cF[:, _FKC:_FKC + 2]

            # ---- GN stats overlap the chained load: per piece Scalar does
            # the sum (Identity + accum into a junk out) and Vector does the
            # sumsq (tensor_tensor_reduce); PE warms (HAM) pace on pieces ----
            sum4 = [sm.tile([128, NXC], f32, tag=f"sum4{t}", name=f"sum4{t}")
                    for t in range(2)]
            sq4 = [sm.tile([128, NXC], f32, tag=f"sq4{t}", name=f"sq4{t}")
                   for t in range(2)]
            stats = [sm.tile([128, 2], f32, tag=f"st{t}", name=f"st{t}")
                     for t in range(2)]
            for cx in range(NXC):
                for t in range(2):
                    sj = sm.tile([128, XC], f16, tag="sjunk", name="sjunk",
                                 bufs=1)
                    nc.scalar.activation(sj[:], xc[t][cx][:], AF.Identity,
                                         accum_out=sum4[t][:, cx:cx + 1])
                    vj = sm.tile([128, XC], f16, tag="vjunk", name="vjunk",
                                 bufs=1)
                    nc.vector.tensor_mul(vj[:], xc[t][cx][:], xc[t][cx][:])
                    nc.vector.reduce_sum(sq4[t][:, cx:cx + 1], vj[:],
                                         axis=AX.X)
                    for r in range(3):
                        w_ps = ps.tile([128, IC], f32, tag="s", name="warm",
                                       bufs=4)
                        nc.tensor.matmul(w_ps[:], ones16,
                                         xc[t][cx][:, (r % 4) * 512:
                                                    (r % 4) * 512 + 512],
                                         start=True, stop=True)
            # dense warm burst on the last piece to flip HAM before the
            # real stream starts
            for r in range(10):
                t, co = r % 2, (r % 4) * 512
                w_ps = ps.tile([128, IC], f32, tag="s", name="warm", bufs=4)
                nc.tensor.matmul(w_ps[:], ones16,
                                 xc[t][NXC - 1][:, co:co + IC],
                                 start=True, stop=True)

            def warmmm(n):
                for r in range(n):
                    t, co = r % 2, (r % 4) * 512
                    w_ps = ps.tile([128, IC], f32, tag="s", name="warm",
                                   bufs=4)
                    nc.tensor.matmul(w_ps[:], ones16,
                                     xc[t][0][:, co:co + IC],
                                     start=True, stop=True)

            # ---- stats combine -> per-channel a, b ----
            for t in range(2):
                nc.vector.reduce_sum(stats[t][:, 0:1], sum4[t][:], axis=AX.X)
                nc.vector.reduce_sum(stats[t][:, 1:2], sq4[t][:], axis=AX.X)
            warmmm(3)
            g_ps = ps.tile([GROUPS, 2], f32, tag="s", name="g_ps", bufs=4)
            for t in range(2):
                nc.tensor.matmul(g_ps[:], gsel[t], stats[t][:],
                                 start=(t == 0), stop=(t == 1))
            for t in range(2):
                f_ps = ps.tile([128, 2], f32, tag="s", name="fill", bufs=4)
                nc.tensor.matmul(f_ps[:], ones_f, stats[t][:],
                                 start=True, stop=True)
                warmmm(2)
            gstats = sm.tile([GROUPS, 2], f32, tag="gstats", name="gstats")
            var = sm.tile([GROUPS, 1], f32, tag="gvar", name="gvar")
            stdt = sm.tile([GROUPS, 1], f32, tag="gstd", name="gstd")
            inv = 1.0 / (CPG * N)
            nc.vector.tensor_scalar_mul(gstats[:, 0:2], g_ps[:, 0:2], inv)
            # negvar = mean*mean - ex2 ; std = sqrt(-negvar + eps)
            nc.vector.scalar_tensor_tensor(var[:], gstats[:, 0:1],
                                           gstats[:, 0:1], gstats[:, 1:2],
                                           op0=OP.mult, op1=OP.subtract)
            nc.scalar.activation(stdt[:], var[:], AF.Sqrt,
                                 bias=kconst[0:GROUPS, 1:2], scale=-1.0)
            nc.vector.reciprocal_approx_fast(gstats[:, 1:2], stdt[:])
            ab = [sm.tile([128, 2], f32, tag=f"ab{t}", name=f"ab{t}")
                  for t in range(2)]
            bcol = [sm.tile([128, 2], f16, tag=f"bcol{t}", name=f"bcol{t}")
                    for t in range(2)]
            for t in range(2):
                bc_ps = ps.tile([128, 2], f32, tag="s", name="bc_ps", bufs=4)
                nc.tensor.matmul(bc_ps[:], gselT[t], gstats[:],
                                 start=True, stop=True)
                # a = rstd*gamma ; b = beta - mean*a
                nc.vector.tensor_mul(ab[t][:, 0:1], bc_ps[:, 1:2], gamma[t])
                nc.vector.tensor_mul(ab[t][:, 1:2], bc_ps[:, 0:1], ab[t][:, 0:1])
                nc.vector.tensor_sub(ab[t][:, 1:2], beta[t], ab[t][:, 1:2])
                nc.vector.tensor_copy(bcol[t][:, 0:1], ab[t][:, 1:2])
                nc.vector.tensor_copy(bcol[t][:, 1:2], ab[t][:, 1:2])

            f_ps = ps.tile([128, 2], f32, tag="s", name="fill", bufs=4)
            nc.tensor.matmul(f_ps[:], ones_f[0:GROUPS, :], gstats[:],
                             start=True, stop=True)
            warmmm(3)

            # ---- scaled moving x: xt = a * x (fp16), chunk 0 first ----
            xt = [[big.tile([128, XC], f16, tag=f"xt{t}_{cx}",
                            name=f"xt{t}_{cx}") for cx in range(NXC)]
                  for t in range(2)]
            for cx in range(NXC):
                for t in range(2):
                    nc.vector.tensor_scalar_mul(xt[t][cx][:], xc[t][cx][:],
                                                ab[t][:, 0:1])
                warmmm(2)

            # ---- on-chip bias columns: cq = Wq b + bq (per attention-channel
            # tile), bfin = WoWv b + (Wo bv + bo) ----
            cqc = [sm.tile([128, 1], f32, tag=f"cq{m}", name=f"cq{m}")
                   for m in range(2)]
            bfin = [sm.tile([128, 1], f32, tag=f"bf{m}", name=f"bf{m}")
                    for m in range(2)]
            for m in range(2):
                c_ps = ps.tile([128, 2], f32, tag="s", name="c_ps", bufs=4)
                for t in range(2):
                    nc.tensor.matmul(c_ps[:], wqT[t][m], bcol[t][:],
                                     start=(t == 0), stop=(t == 1))
                nc.vector.tensor_scalar_add(cqc[m][:], c_ps[:, 0:1], bq[m])
            for m in range(2):
                c_ps = ps.tile([128, 2], f32, tag="s", name="c_ps", bufs=4)
                for t in range(2):
                    nc.tensor.matmul(c_ps[:], wovT[t][:, m * 128:m * 128 + 128],
                                     bcol[t][:], start=(t == 0), stop=(t == 1))
                nc.vector.tensor_scalar_add(bfin[m][:], c_ps[:, 0:1], wobvbo[m])
            warmmm(2)

            # ---- projections on scaled x ----
            qt = [[big.tile([128, IC], f32r, tag=f"q{e}_{f}", name=f"q{e}_{f}")
                   for f in range(NIC)] for e in range(2)]
            kt = [[big.tile([128, 512], f32r, tag=f"k{e}_{f}", name=f"k{e}_{f}")
                   for f in range(N // 512)] for e in range(2)]
            # vv pairs: [vv(2u) | vv(2u+1)], each [128 keys, 256 ch] bf16
            vvp = [big.tile([128, 512], bf16, tag=f"vv{u}", name=f"vv{u}")
                   for u in range(NJT // 2)]

            def qproj(f):
                for e in range(2):
                    q_ps = ps.tile([128, IC], f32, tag="s", name="q_ps",
                                   bufs=4)
                    for t in range(2):
                        nc.tensor.matmul(
                            q_ps[:],
                            cR[:, _RQ + 256 * t + 128 * e:
                               _RQ + 256 * t + 128 * e + 128],
                            xt[t][0][:, f * 512:(f + 1) * 512],
                            start=(t == 0), stop=(t == 1))
                    nc.vector.tensor_scalar_add(qt[e][f][:], q_ps[:], cqc[e])

            def kproj(f):
                for e in range(2):
                    k_ps = ps.tile([128, 512], f32, tag="s", name="k_ps",
                                   bufs=4)
                    for t in range(2):
                        nc.tensor.matmul(
                            k_ps[:],
                            cR[:, _RK + 256 * t + 128 * e:
                               _RK + 256 * t + 128 * e + 128],
                            xt[t][f // 4][:, (f % 4) * 512:(f % 4 + 1) * 512],
                            start=(t == 0), stop=(t == 1))
                    nc.scalar.activation(kt[e][f][:], k_ps[:],
                                         AF.Identity)

            def vvproj(u):
                v_ps = ps.tile([128, 512], f32, tag="s", name="v_ps", bufs=4)
                for jj in range(2):
                    jt = 2 * u + jj
                    for t in range(2):
                        nc.tensor.matmul(
                            v_ps[:, jj * 256:jj * 256 + 256],
                            xt[t][jt // 16][:, (jt % 16) * 128:
                                            (jt % 16) * 128 + 128],
                            wovT[t], start=(t == 0), stop=(t == 1))
                nc.scalar.activation(vvp[u][:], v_ps[:], AF.Identity)

            kproj(0)
            kproj(1)
            qproj(0)
            qproj(1)
            vvproj(0)

            # ---- attention: 2 supers x 2 chunks x 32 key tiles ----
            lacc = [resp.tile([128, IC], f32r, tag=f"lacc{c}", name=f"lacc{c}",
                              bufs=1) for c in range(NIC)]
            osb = [[None, None] for c in range(NIC)]
            rx = [[None, None] for c in range(NIC)]
            plast = {}
            rbt = {}

            def make_rx(c):
                for m in range(2):
                    r = resp.tile([128, IC], f32, tag=f"rx{c}_{m}",
                                  name=f"rx{c}_{m}", bufs=1)
                    nc.vector.tensor_scalar_add(
                        r[:], xc[m][0][:, c * IC:(c + 1) * IC], bfin[m])
                    rx[c][m] = r

            def denom(c):
                # softmax denominator for chunk c -> reciprocal rbt[c]
                lbc_ps = ps.tile([128, IC], f32, tag="s", name="lbc_ps",
                                 bufs=4)
                nc.tensor.matmul(lbc_ps[:], ones128, lacc[c][:],
                                 start=True, stop=False)
                p30, p31 = plast[c]
                nc.tensor.matmul(lbc_ps[:], ones_h, p30[:],
                                 start=False, stop=False)
                nc.tensor.matmul(lbc_ps[:], ones_h, p31[:],
                                 start=False, stop=True)
                rb = resp.tile([128, IC], f32, tag=f"rb{c}", name=f"rb{c}",
                               bufs=1)
                nc.vector.reciprocal_approx_fast(rb[:], lbc_ps[:])
                rbt[c] = rb

            def finalize0(sc):
                # deferred finalize for super-0 chunks (hidden mid-stream):
                # GpSimd adds, merged [128,1024] out DMA per channel-half
                denom(2 * sc)
                denom(2 * sc + 1)
                for m in range(2):
                    res = resp.tile([128, 1024], bf16, tag=f"res0_{m}",
                                    name=f"res0_{m}", bufs=1)
                    for ci in range(2):
                        c = 2 * sc + ci
                        scaled = resp.tile([128, IC], f32, tag="scaled",
                                           name="scaled")
                        nc.vector.tensor_mul(scaled[:], osb[c][m][:],
                                             rbt[c][:])
                        nc.gpsimd.tensor_add(res[:, ci * IC:(ci + 1) * IC],
                                             scaled[:], rx[c][m][:])
                    nc.sync.dma_start(out_e[m, :, 2 * sc * IC:
                                            (2 * sc + 2) * IC], res[:])

            for sc in range(2):
                ca, cb = 2 * sc, 2 * sc + 1
                pv_ps = [[ps.tile([128, IC], f32, tag=f"pv{ci}_{m}",
                                  name=f"pv{ci}_{m}", bufs=1)
                          for m in range(2)] for ci in range(2)]

                def scores_block(jt):
                    s_a = ps.tile([128, IC], f32, tag="s", name="s_a", bufs=4)
                    s_b = ps.tile([128, IC], f32, tag="s", name="s_b", bufs=4)
                    for e in range(2):
                        ktile = kt[e][jt // 4][:, (jt % 4) * 128:(jt % 4 + 1) * 128]
                        nc.tensor.matmul(s_a[:], ktile, qt[e][ca][:],
                                         start=(e == 0), stop=(e == 1))
                        nc.tensor.matmul(s_b[:], ktile, qt[e][cb][:],
                                         start=(e == 0), stop=(e == 1))
                    ptag = "pt" if jt < NJT - 2 else "pfin"
                    pbufs = {} if jt < NJT - 2 else {"bufs": 4}
                    p_a = ptp.tile([128, IC], bf16, tag=ptag, name=ptag,
                                   **pbufs)
                    nc.scalar.activation(p_a[:], s_a[:], AF.Exp,
                                         bias=kconst[:, 0:1])
                    p_b = ptp.tile([128, IC], bf16, tag=ptag, name=ptag,
                                   **pbufs)
                    nc.scalar.activation(p_b[:], s_b[:], AF.Exp,
                                         bias=kconst[:, 0:1])
                    if jt == NJT - 2:
                        plast[ca] = [p_a, None]
                        plast[cb] = [p_b, None]
                    elif jt == NJT - 1:
                        plast[ca][1] = p_a
                        plast[cb][1] = p_b
                    return p_a, p_b

                # software pipeline: scores/exp emitted one jt ahead of PV
                # so the PE queue never stalls waiting on the exp latency
                p_next = scores_block(0)
                for jt in range(NJT):
                    p_a, p_b = p_next
                    if jt + 1 < NJT:
                        p_next = scores_block(jt + 1)
                    if sc == 1 and jt == NJT - 1:
                        # hoist the denominators + reciprocals ahead of the
                        # final PV matmuls so only mul/add/DMA trail the PE
                        denom(ca)
                        denom(cb)
                    for m in range(2):
                        vslice = vvp[jt // 2][:, (jt % 2) * 256 + m * 128:
                                              (jt % 2) * 256 + m * 128 + 128]
                        nc.tensor.matmul(pv_ps[0][m][:], vslice, p_a[:],
                                         start=(jt == 0), stop=(jt == NJT - 1))
                        nc.tensor.matmul(pv_ps[1][m][:], vslice, p_b[:],
                                         start=(jt == 0), stop=(jt == NJT - 1))
                    if sc == 0:
                        if jt % 4 == 0 and 4 <= jt <= 24:
                            kproj(jt // 4 + 1)
                        if jt == 2:
                            qproj(2)
                        if jt == 6:
                            qproj(3)
                        if jt % 2 == 0 and jt < NJT - 2:
                            vvproj(jt // 2 + 1)
                    if jt == 0:
                        nc.vector.tensor_copy(lacc[ca][:], p_a[:])
                        nc.gpsimd.tensor_copy(lacc[cb][:], p_b[:])
                    elif jt < NJT - 2:
                        nc.vector.tensor_add(lacc[ca][:], lacc[ca][:], p_a[:])
                        nc.gpsimd.tensor_add(lacc[cb][:], lacc[cb][:], p_b[:])
                    if sc == 1 and jt == 6:
                        finalize0(0)
                    if sc == 1 and jt == 20:
                        make_rx(2)
                        make_rx(3)
                if sc == 0:
                    make_rx(0)
                    make_rx(1)
                    # evacuate pv PSUM so super1 can reuse the banks
                    for ci in range(2):
                        for m in range(2):
                            o = resp.tile([128, IC], f32r, tag=f"osb{ci}_{m}",
                                          name=f"osb{ci}_{m}", bufs=1)
                            nc.vector.tensor_copy(o[:], pv_ps[ci][m][:])
                            osb[2 * sc + ci][m] = o
                else:
                    for r in range(16):
                        w_ps = ps.tile([128, IC], f32, tag="s", name="warm",
                                       bufs=4)
                        nc.tensor.matmul(w_ps[:], ones16, jk[:], start=True,
                                         stop=True)
                    # closing finalize: Vector does the PSUM-reading muls
                    # (GpSimd cannot access PSUM) + m=0 adds; GpSimd does the
                    # m=1 adds; bf16 res halves the exposed out-DMA bytes and
                    # each m's merged [128,1024] DMA fires right after its
                    # own adds
                    scl = [[resp.tile([128, IC], f32, tag=f"scl{ci}_{m}",
                                      name=f"scl{ci}_{m}", bufs=1)
                            for m in range(2)] for ci in range(2)]
                    for ci in range(2):
                        for m in range(2):
                            nc.vector.tensor_mul(scl[ci][m][:],
                                                 pv_ps[ci][m][:],
                                                 rbt[2 * sc + ci][:])
                    res1 = [resp.tile([128, 1024], bf16, tag=f"res1_{m}",
                                      name=f"res1_{m}", bufs=1)
                            for m in range(2)]
                    for ci in range(2):
                        nc.vector.tensor_add(
                            res1[0][:, ci * IC:(ci + 1) * IC],
                            scl[ci][0][:], rx[2 * sc + ci][0][:])
                    nc.sync.dma_start(out_e[0, :, 2 * sc * IC:
                                            (2 * sc + 2) * IC], res1[0][:])
                    for ci in range(2):
                        nc.gpsimd.tensor_add(
                            res1[1][:, ci * IC:(ci + 1) * IC],
                            scl[ci][1][:], rx[2 * sc + ci][1][:])
                    nc.sync.dma_start(out_e[1, :, 2 * sc * IC:
                                            (2 * sc + 2) * IC], res1[1][:])

    nc.compile()
    return nc


def _prep_inputs(x, gn_gamma, gn_beta, wq, bq, wk, bk, wv, bv, wo, bo):
    f = np.float32
    constR = np.zeros((128, _RCOLS), np.float16)
    wov = (wo.astype(f) @ wv.astype(f))
    for base, w in ((_RQ, wq), (_RK, wk), (_RO, wov)):
        wT = w.astype(f).T  # [c_in, c_out]
        for t in range(2):
            constR[:, base + 256 * t:base + 256 * t + 256] = \
                wT[128 * t:128 * (t + 1), :]
    constF = np.zeros((128, _FCOLS), f)
    gsel = np.zeros((2, 128, GROUPS), f)
    gselT = np.zeros((2, GROUPS, 128), f)
    for t in range(2):
        for p in range(128):
            g = (t * 128 + p) // CPG
            gsel[t, p, g] = 1.0
            gselT[t, g, p] = 1.0
    for t in range(2):
        constF[:, _FGS + 32 * t:_FGS + 32 * t + 32] = gsel[t]
        constF[0:GROUPS, _FGT + 128 * t:_FGT + 128 * t + 128] = gselT[t]
    wobvbo = (wo.astype(f) @ bv.astype(f) + bo.astype(f))
    vecs = (gn_gamma, gn_beta, bq, wobvbo)
    for i, v in enumerate(vecs):
        vv = v.astype(f).reshape(2, 128)
        for t in range(2):
            constF[:, _FVEC + 2 * i + t] = vv[t]
    constF[:, _FKC + 0] = -SHIFT
    constF[:, _FKC + 1] = EPS

    common = dict(constR=constR, constF=constF)
    xb = x.reshape(B, C, N).astype(np.float16)
    in_maps = []
    for core in range(NCORES):
        bi, qh = core // 2, core % 2
        xcore = xb[bi]
        if qh:
            xcore = np.concatenate([xcore[:, NQ:], xcore[:, :NQ]], axis=1)
        in_maps.append(dict(x=np.ascontiguousarray(xcore.reshape(2, 128, N)),
                            **common))
    return in_maps


def _execute(inputs, trace=False, **kw):
    from concourse.bass_utils import run_bass_kernel_spmd
    if "nc" not in _cache:
        _cache["nc"] = _build()
    nc = _cache["nc"]
    in_maps = _prep_inputs(**inputs)
    res = run_bass_kernel_spmd(nc, in_maps, core_ids=list(range(NCORES)),
                               trace=trace, **kw)
    out = np.empty((B, C, N), np.float32)
    for core in range(NCORES):
        bi, qh = core // 2, core % 2
        chunk = np.asarray(res.results[core]["out"]).astype(
            np.float32).reshape(C, NQ)
        out[bi, :, qh * NQ:(qh + 1) * NQ] = chunk
    return out.reshape(B, C, DD, HH, WW), res


def kernel(**inputs):
    out, _ = _execute(inputs, trace=False)
    return out


# revision 16
# speedup vs baseline: 1.1304x; 1.0411x over previous
"""AttnBlock3d on 8 TRN2 NeuronCores.

Sharding: 8 cores = 4 batches x 2 query-halves. Each core receives its
batch's full x (rotated so its query half is always voxels [0:2048] --
GroupNorm and the attention key-reduction are voxel-permutation
invariant, so all cores run an identical graph), computes GN stats +
QKV + full attention for its 2048 queries and writes a [2,128,2048]
channel-tiled chunk.

Math restructuring vs the reference (exact up to fp rounding):
- x is shipped as fp16 (halves the HBM load; fp16 keeps 11 mantissa
  bits so the scores stay accurate, unlike bf16).  The GN apply is
  folded into a per-channel scale of the MOVING x: xt = a*x with
  a = gamma*rstd; projections use the RAW f32r transposed weights as
  stationaries (mixed fp16 x bass-matmul).  The b-terms reduce to
  per-query score shifts (softmax-invariant, dropped) plus the q bias
  cq = Wq b + bq (tiny on-chip matmuls); k needs no bias at all.
- The output projection is fused into V: vv = (Wo Wv) xt (Wo@Wv
  precomputed on host) so PV directly produces the o-projected output;
  the remaining constant (WoWv b + Wo bv + bo) is added at the end.
- Scores are computed transposed (S^T = k^T q, f32r) so no transposes
  are needed; exp applies a constant shift (scores stay in [-97,97]).
  The softmax denominator accumulates P^T tiles on Vector/GpSimd,
  summed across partitions by a ones-matmul; normalization is applied
  at the very end (linearity).
- The x load is CHAINED (tiny guard-DMAs on the Sync queue serialize
  the piece triggers) so pieces arrive in order and the GN stats
  (Scalar: sum-accum; Vector: tensor_tensor_reduce sumsq) overlap the
  load instead of serializing after it.
- The attention loop processes TWO 512-query chunks per key-tile pass
  so each stationary tile serves 2 matmuls.
- The last super's softmax denominators + reciprocals are hoisted
  before the final PV matmuls; the closing normalize+residual is split
  Vector/GpSimd and the output DMA is merged to one [128,1024] write
  per channel-half, so almost nothing trails the last matmul.
- The DMA-load head warms the PE (HAM) with junk matmuls paced by x
  piece arrivals, and activation tables (Identity/Sqrt/Exp) are
  preloaded with dummies so no table load lands on the critical path.
"""

import sys

for _p in ("/opt/trn_rl_repo",):
    if _p not in sys.path:
        sys.path.append(_p)

import numpy as np

B, C, DD, HH, WW = 4, 256, 16, 16, 16
N = DD * HH * WW          # 4096 voxels
NQ = N // 2               # queries per core
GROUPS = 32
CPG = C // GROUPS         # channels per group
EPS = 1e-6
SHIFT = 60.0              # softmax constant shift
NCORES = 8
IC = 512                  # query chunk
NIC = NQ // IC            # 4 chunks
NJT = N // 128            # 32 key tiles
XC = 2048                 # x-load / GN piece (voxels)
NXC = N // XC             # 2 pieces per channel-half

# packed-constant column offsets (constR: f32r, constF: f32)
_RQ, _RK, _RO = 0, 512, 1024
_RCOLS = 1536
_FGS, _FGT, _FVEC, _FKC = 0, 64, 320, 328
_FCOLS = 330

CHAIN = 1  # serialize x piece DMAs via guard-DMAs on the Sync queue

_cache = {}


def _build():
    import concourse.bass as bass
    from concourse import bacc, mybir, tile

    f32 = mybir.dt.float32
    f32r = mybir.dt.float32r
    f16 = mybir.dt.float16
    bf16 = mybir.dt.bfloat16
    AF = mybir.ActivationFunctionType
    OP = mybir.AluOpType
    AX = mybir.AxisListType

    nc = bacc.Bacc("TRN2", target_bir_lowering=False, debug=False,
                   num_devices=NCORES)

    x_e = nc.dram_tensor("x", [2, 128, N], f16, kind="ExternalInput").ap()
    cR_e = nc.dram_tensor("constR", [128, _RCOLS], f16,
                          kind="ExternalInput").ap()
    cF_e = nc.dram_tensor("constF", [128, _FCOLS], f32,
                          kind="ExternalInput").ap()
    out_e = nc.dram_tensor("out", [2, 128, NQ], bf16,
                           kind="ExternalOutput").ap()

    with tile.TileContext(nc) as tc:
        with tc.tile_pool(name="big", bufs=1) as big, \
             tc.tile_pool(name="w", bufs=1) as wp, \
             tc.tile_pool(name="sm", bufs=2) as sm, \
             tc.tile_pool(name="pt", bufs=12) as ptp, \
             tc.tile_pool(name="res", bufs=2) as resp, \
             tc.tile_pool(name="psum", bufs=1, space="PSUM") as ps:

            # ---- on-chip constants (no DMA dependency) ----
            ones128 = wp.tile([128, 128], f32r, tag="ones", name="ones128")
            ones_h = wp.tile([128, 128], bf16, tag="onesh", name="ones_h")
            ones_f = wp.tile([128, 128], f32, tag="onesf", name="ones_f")
            ones16 = wp.tile([128, 128], f16, tag="ones16", name="ones16")
            nc.vector.memset(ones_f[:], 1.0)
            nc.vector.tensor_copy(ones128[:], ones_f[:])
            nc.vector.tensor_copy(ones_h[:], ones_f[:])
            nc.vector.tensor_copy(ones16[:], ones_f[:])
            dzi = sm.tile([GROUPS, 1], f32, tag="dzi", name="dzi")
            dzo = sm.tile([GROUPS, 1], f32, tag="dzo", name="dzo")
            nc.vector.memset(dzi[:], 0.25)
            # preload the activation tables off the critical path
            nc.scalar.activation(dzo[:], dzi[:], AF.Square)
            nc.scalar.activation(dzo[:], dzi[:], AF.Sqrt,
                                 bias=dzi[:], scale=-1.0)
            nc.scalar.activation(dzo[:], dzi[:], AF.Exp, bias=dzi[:])
            # pre-warm the engines (DVFS ramps after ~3us of activity): PE
            # warms on a constant junk tile until x arrives; Vector/GpSimd
            # run junk ops so the stats hit full-rate engines
            jk = wp.tile([128, 512], f16, tag="jk", name="jk")
            jk2 = wp.tile([128, 512], f16, tag="jk2", name="jk2")
            nc.vector.memset(jk[:], 1.0)
            for r in range(6):
                nc.vector.tensor_copy(jk2[:], jk[:])
            for r in range(2):
                nc.gpsimd.tensor_copy(jk2[:], jk[:])
            for r in range(10):
                w_ps = ps.tile([128, IC], f32, tag="s", name="warm", bufs=4)
                nc.tensor.matmul(w_ps[:], ones16, jk[:], start=True,
                                 stop=True)

            # ---- chained x load: pieces arrive IN ORDER so GN stats
            # overlap the load.  Tiny guard-DMAs on the Sync queue make the
            # engine wait for a piece before triggering later pieces. ----
            xc = [[big.tile([128, XC], f16, tag=f"x{t}_{cx}", name=f"x{t}_{cx}")
                   for cx in range(NXC)] for t in range(2)]
            cR = wp.tile([128, _RCOLS], f16, tag="cR", name="cR")
            cF = wp.tile([128, _FCOLS], f32, tag="cF", name="cF")
            gdst = sm.tile([1, 16], f16, tag="gdst", name="gdst")

            def trig(t, cx):
                sl = slice(cx * XC, (cx + 1) * XC)
                nc.sync.dma_start(xc[t][cx][:], x_e[t, :, sl])

            def guard(t, cx):
                nc.sync.dma_start(gdst[:], xc[t][cx][0:1, 0:16])

            # piece order: (0,0) (1,0) (0,1) (1,1); first piece rides
            # alone for the earliest stats start, then 2-deep pipeline
            if CHAIN:
                trig(0, 0)
                guard(0, 0)
                trig(1, 0)
                trig(0, 1)
                guard(1, 0)
                trig(1, 1)
                nc.sync.dma_start(cF[:], cF_e[:])
                guard(0, 1)
                nc.sync.dma_start(cR[:], cR_e[:])
            else:
                trig(0, 0)
                trig(1, 0)
                nc.sync.dma_start(cF[:], cF_e[:])
                trig(0, 1)
                trig(1, 1)
                nc.sync.dma_start(cR[:], cR_e[:])

            # raw transposed weights [c_in, c_out]; blocks (2t+m)
            wqT = [[cR[:, _RQ + 128 * (2 * t + m):_RQ + 128 * (2 * t + m) + 128]
                    for m in range(2)] for t in range(2)]
            wovT = [cR[:, _RO + 256 * t:_RO + 256 * t + 256] for t in range(2)]
            gsel = [cF[:, _FGS + 32 * t:_FGS + 32 * t + 32] for t in range(2)]
            gselT = [cF[0:GROUPS, _FGT + 128 * t:_FGT + 128 * t + 128]
                     for t in range(2)]
            gamma = [cF[:, _FVEC + 0 + t:_FVEC + 1 + t] for t in range(2)]
            beta = [cF[:, _FVEC + 2 + t:_FVEC + 3 + t] for t in range(2)]
            bq = [cF[:, _FVEC + 4 + t:_FVEC + 5 + t] for t in range(2)]
            wobvbo = [cF[:, _FVEC + 6 + t:_FVEC + 7 + t] for t in range(2)]
            kconst = cF[:, _FKC:_FKC + 2]

            # ---- GN stats overlap the chained load: per piece Scalar does
            # the sum (Identity + accum into a junk out) and Vector does the
            # sumsq (tensor_tensor_reduce); PE warms (HAM) pace on pieces ----
            sum4 = [sm.tile([128, NXC], f32, tag=f"sum4{t}", name=f"sum4{t}")
                    for t in range(2)]
            sq4 = [sm.tile([128, NXC], f32, tag=f"sq4{t}", name=f"sq4{t}")
                   for t in range(2)]
            stats = [sm.tile([128, 2], f32, tag=f"st{t}", name=f"st{t}")
                     for t in range(2)]
            for cx in range(NXC):
                for t in range(2):
                    nc.vector.reduce_sum(sum4[t][:, cx:cx + 1],
                                         xc[t][cx][:], axis=AX.X)
                    vj = sm.tile([128, XC], f16, tag="vjunk", name="vjunk",
                                 bufs=1)
                    nc.scalar.activation(vj[:], xc[t][cx][:], AF.Square,
                                         accum_out=sq4[t][:, cx:cx + 1])
                    for r in range(3):
                        w_ps = ps.tile([128, IC], f32, tag="s", name="warm",
                                       bufs=4)
                        nc.tensor.matmul(w_ps[:], ones16,
                                         xc[t][cx][:, (r % 4) * 512:
                                                    (r % 4) * 512 + 512],
                                         start=True, stop=True)
            # dense warm burst on the last piece to flip HAM before the
            # real stream starts
            for r in range(10):
                t, co = r % 2, (r % 4) * 512
                w_ps = ps.tile([128, IC], f32, tag="s", name="warm", bufs=4)
                nc.tensor.matmul(w_ps[:], ones16,
                                 xc[t][NXC - 1][:, co:co + IC],
                                 start=True, stop=True)

            def warmmm(n):
                for r in range(n):
                    t, co = r % 2, (r % 4) * 512
                    w_ps = ps.tile([128, IC], f32, tag="s", name="warm",
                                   bufs=4)
                    nc.tensor.matmul(w_ps[:], ones16,
                                     xc[t][0][:, co:co + IC],
                                     start=True, stop=True)

            # ---- stats combine -> per-channel a, b ----
            for t in range(2):
                nc.vector.reduce_sum(stats[t][:, 0:1], sum4[t][:], axis=AX.X)
                nc.vector.reduce_sum(stats[t][:, 1:2], sq4[t][:], axis=AX.X)
            warmmm(3)
            g_ps = ps.tile([GROUPS, 2], f32, tag="s", name="g_ps", bufs=4)
            for t in range(2):
                nc.tensor.matmul(g_ps[:], gsel[t], stats[t][:],
                                 start=(t == 0), stop=(t == 1))
            for t in range(2):
                f_ps = ps.tile([128, 2], f32, tag="s", name="fill", bufs=4)
                nc.tensor.matmul(f_ps[:], ones_f, stats[t][:],
                                 start=True, stop=True)
                warmmm(2)
            gstats = sm.tile([GROUPS, 2], f32, tag="gstats", name="gstats")
            var = sm.tile([GROUPS, 1], f32, tag="gvar", name="gvar")
            stdt = sm.tile([GROUPS, 1], f32, tag="gstd", name="gstd")
            inv = 1.0 / (CPG * N)
            nc.vector.tensor_scalar_mul(gstats[:, 0:2], g_ps[:, 0:2], inv)
            # negvar = mean*mean - ex2 ; std = sqrt(-negvar + eps)
            nc.vector.scalar_tensor_tensor(var[:], gstats[:, 0:1],
                                           gstats[:, 0:1], gstats[:, 1:2],
                                           op0=OP.mult, op1=OP.subtract)
            nc.scalar.activation(stdt[:], var[:], AF.Sqrt,
                                 bias=kconst[0:GROUPS, 1:2], scale=-1.0)
            nc.vector.reciprocal_approx_fast(gstats[:, 1:2], stdt[:])
            ab = [sm.tile([128, 2], f32, tag=f"ab{t}", name=f"ab{t}")
                  for t in range(2)]
            bcol = [sm.tile([128, 2], f16, tag=f"bcol{t}", name=f"bcol{t}")
                    for t in range(2)]
            for t in range(2):
                bc_ps = ps.tile([128, 2], f32, tag="s", name="bc_ps", bufs=4)
                nc.tensor.matmul(bc_ps[:], gselT[t], gstats[:],
                                 start=True, stop=True)
                # a = rstd*gamma ; b = beta - mean*a
                nc.vector.tensor_mul(ab[t][:, 0:1], bc_ps[:, 1:2], gamma[t])
                nc.vector.tensor_mul(ab[t][:, 1:2], bc_ps[:, 0:1], ab[t][:, 0:1])
                nc.vector.tensor_sub(ab[t][:, 1:2], beta[t], ab[t][:, 1:2])
                nc.vector.tensor_copy(bcol[t][:, 0:1], ab[t][:, 1:2])
                nc.vector.tensor_copy(bcol[t][:, 1:2], ab[t][:, 1:2])

            f_ps = ps.tile([128, 2], f32, tag="s", name="fill", bufs=4)
            nc.tensor.matmul(f_ps[:], ones_f[0:GROUPS, :], gstats[:],
                             start=True, stop=True)
            warmmm(3)

            # ---- fold a into the weights: W* = W diag(a) (row-scale of
            # the fp16 transposed tiles; 8x less data than scaling x) ----
            wks = [wp.tile([128, 256], f16, tag=f"wks{t}", name=f"wks{t}")
                   for t in range(2)]
            wqs = [wp.tile([128, 256], f16, tag=f"wqs{t}", name=f"wqs{t}")
                   for t in range(2)]
            wovs = [wp.tile([128, 256], f16, tag=f"wovs{t}", name=f"wovs{t}")
                    for t in range(2)]
            for t in range(2):
                nc.vector.tensor_scalar_mul(
                    wks[t][:], cR[:, _RK + 256 * t:_RK + 256 * t + 256],
                    ab[t][:, 0:1])
            for t in range(2):
                nc.vector.tensor_scalar_mul(
                    wqs[t][:], cR[:, _RQ + 256 * t:_RQ + 256 * t + 256],
                    ab[t][:, 0:1])
            for t in range(2):
                nc.vector.tensor_scalar_mul(wovs[t][:], wovT[t],
                                            ab[t][:, 0:1])
            warmmm(2)

            # ---- on-chip bias columns: cq = Wq b + bq (per attention-channel
            # tile), bfin = WoWv b + (Wo bv + bo) ----
            cqc = [sm.tile([128, 1], f32, tag=f"cq{m}", name=f"cq{m}")
                   for m in range(2)]
            bfin = [sm.tile([128, 1], f32, tag=f"bf{m}", name=f"bf{m}")
                    for m in range(2)]
            for m in range(2):
                c_ps = ps.tile([128, 2], f32, tag="s", name="c_ps", bufs=4)
                for t in range(2):
                    nc.tensor.matmul(c_ps[:], wqT[t][m], bcol[t][:],
                                     start=(t == 0), stop=(t == 1))
                nc.vector.tensor_scalar_add(cqc[m][:], c_ps[:, 0:1], bq[m])
            for m in range(2):
                c_ps = ps.tile([128, 2], f32, tag="s", name="c_ps", bufs=4)
                for t in range(2):
                    nc.tensor.matmul(c_ps[:], wovT[t][:, m * 128:m * 128 + 128],
                                     bcol[t][:], start=(t == 0), stop=(t == 1))
                nc.vector.tensor_scalar_add(bfin[m][:], c_ps[:, 0:1], wobvbo[m])
            warmmm(2)

            # ---- projections on scaled x ----
            qt = [[big.tile([128, IC], f32r, tag=f"q{e}_{f}", name=f"q{e}_{f}")
                   for f in range(NIC)] for e in range(2)]
            kt = [[big.tile([128, 512], f32r, tag=f"k{e}_{f}", name=f"k{e}_{f}")
                   for f in range(N // 512)] for e in range(2)]
            # vv pairs: [vv(2u) | vv(2u+1)], each [128 keys, 256 ch] bf16
            vvp = [big.tile([128, 512], bf16, tag=f"vv{u}", name=f"vv{u}")
                   for u in range(NJT // 2)]

            def qproj(f):
                for e in range(2):
                    q_ps = ps.tile([128, IC], f32, tag="s", name="q_ps",
                                   bufs=4)
                    for t in range(2):
                        nc.tensor.matmul(
                            q_ps[:], wqs[t][:, e * 128:e * 128 + 128],
                            xc[t][0][:, f * 512:(f + 1) * 512],
                            start=(t == 0), stop=(t == 1))
                    nc.vector.tensor_scalar_add(qt[e][f][:], q_ps[:], cqc[e])

            def kproj(f):
                for e in range(2):
                    k_ps = ps.tile([128, 512], f32, tag="s", name="k_ps",
                                   bufs=4)
                    for t in range(2):
                        nc.tensor.matmul(
                            k_ps[:], wks[t][:, e * 128:e * 128 + 128],
                            xc[t][f // 4][:, (f % 4) * 512:(f % 4 + 1) * 512],
                            start=(t == 0), stop=(t == 1))
                    nc.scalar.activation(kt[e][f][:], k_ps[:],
                                         AF.Identity)

            def vvproj(u):
                v_ps = ps.tile([128, 512], f32, tag="s", name="v_ps", bufs=4)
                for jj in range(2):
                    jt = 2 * u + jj
                    for t in range(2):
                        nc.tensor.matmul(
                            v_ps[:, jj * 256:jj * 256 + 256],
                            xc[t][jt // 16][:, (jt % 16) * 128:
                                            (jt % 16) * 128 + 128],
                            wovs[t], start=(t == 0), stop=(t == 1))
                nc.scalar.activation(vvp[u][:], v_ps[:], AF.Identity)

            kproj(0)
            kproj(1)
            qproj(0)
            qproj(1)
            vvproj(0)

            # ---- attention: 2 supers x 2 chunks x 32 key tiles ----
            lacc = [resp.tile([128, IC], f32r, tag=f"lacc{c}", name=f"lacc{c}",
                              bufs=1) for c in range(NIC)]
            osb = [[None, None] for c in range(NIC)]
            rx = [[None, None] for c in range(NIC)]
            plast = {}
            rbt = {}

            def make_rx(c):
                for m in range(2):
                    r = resp.tile([128, IC], f32, tag=f"rx{c}_{m}",
                                  name=f"rx{c}_{m}", bufs=1)
                    nc.vector.tensor_scalar_add(
                        r[:], xc[m][0][:, c * IC:(c + 1) * IC], bfin[m])
                    rx[c][m] = r

            def denom(c):
                # softmax denominator for chunk c -> reciprocal rbt[c]
                lbc_ps = ps.tile([128, IC], f32, tag="s", name="lbc_ps",
                                 bufs=4)
                nc.tensor.matmul(lbc_ps[:], ones128, lacc[c][:],
                                 start=True, stop=False)
                p30, p31 = plast[c]
                nc.tensor.matmul(lbc_ps[:], ones_h, p30[:],
                                 start=False, stop=False)
                nc.tensor.matmul(lbc_ps[:], ones_h, p31[:],
                                 start=False, stop=True)
                rb = resp.tile([128, IC], f32, tag=f"rb{c}", name=f"rb{c}",
                               bufs=1)
                nc.vector.reciprocal_approx_fast(rb[:], lbc_ps[:])
                rbt[c] = rb

            def finalize0(sc):
                # deferred finalize for super-0 chunks (hidden mid-stream):
                # GpSimd adds, merged [128,1024] out DMA per channel-half
                denom(2 * sc)
                denom(2 * sc + 1)
                for m in range(2):
                    res = resp.tile([128, 1024], bf16, tag=f"res0_{m}",
                                    name=f"res0_{m}", bufs=1)
                    for ci in range(2):
                        c = 2 * sc + ci
                        scaled = resp.tile([128, IC], f32, tag="scaled",
                                           name="scaled")
                        nc.vector.tensor_mul(scaled[:], osb[c][m][:],
                                             rbt[c][:])
                        nc.gpsimd.tensor_add(res[:, ci * IC:(ci + 1) * IC],
                                             scaled[:], rx[c][m][:])
                    nc.sync.dma_start(out_e[m, :, 2 * sc * IC:
                                            (2 * sc + 2) * IC], res[:])

            for sc in range(2):
                ca, cb = 2 * sc, 2 * sc + 1
                pv_ps = [[ps.tile([128, IC], f32, tag=f"pv{ci}_{m}",
                                  name=f"pv{ci}_{m}", bufs=1)
                          for m in range(2)] for ci in range(2)]

                def scores_block(jt):
                    s_a = ps.tile([128, IC], f32, tag="s", name="s_a", bufs=4)
                    s_b = ps.tile([128, IC], f32, tag="s", name="s_b", bufs=4)
                    for e in range(2):
                        ktile = kt[e][jt // 4][:, (jt % 4) * 128:(jt % 4 + 1) * 128]
                        nc.tensor.matmul(s_a[:], ktile, qt[e][ca][:],
                                         start=(e == 0), stop=(e == 1))
                        nc.tensor.matmul(s_b[:], ktile, qt[e][cb][:],
                                         start=(e == 0), stop=(e == 1))
                    ptag = "pt" if jt < NJT - 2 else "pfin"
                    pbufs = {} if jt < NJT - 2 else {"bufs": 4}
                    p_a = ptp.tile([128, IC], bf16, tag=ptag, name=ptag,
                                   **pbufs)
                    nc.scalar.activation(p_a[:], s_a[:], AF.Exp,
                                         bias=kconst[:, 0:1])
                    p_b = ptp.tile([128, IC], bf16, tag=ptag, name=ptag,
                                   **pbufs)
                    nc.scalar.activation(p_b[:], s_b[:], AF.Exp,
                                         bias=kconst[:, 0:1])
                    if jt == NJT - 2:
                        plast[ca] = [p_a, None]
                        plast[cb] = [p_b, None]
                    elif jt == NJT - 1:
                        plast[ca][1] = p_a
                        plast[cb][1] = p_b
                    return p_a, p_b

                # software pipeline: scores/exp emitted one jt ahead of PV
                # so the PE queue never stalls waiting on the exp latency
                p_next = scores_block(0)
                for jt in range(NJT):
                    p_a, p_b = p_next
                    if jt + 1 < NJT:
                        p_next = scores_block(jt + 1)
                    if sc == 1 and jt == NJT - 1:
                        # hoist the denominators + reciprocals ahead of the
                        # final PV matmuls so only mul/add/DMA trail the PE
                        denom(ca)
                        denom(cb)
                    for m in range(2):
                        vslice = vvp[jt // 2][:, (jt % 2) * 256 + m * 128:
                                              (jt % 2) * 256 + m * 128 + 128]
                        nc.tensor.matmul(pv_ps[0][m][:], vslice, p_a[:],
                                         start=(jt == 0), stop=(jt == NJT - 1))
                        nc.tensor.matmul(pv_ps[1][m][:], vslice, p_b[:],
                                         start=(jt == 0), stop=(jt == NJT - 1))
                    if sc == 0:
                        if jt % 4 == 0 and 4 <= jt <= 24:
                            kproj(jt // 4 + 1)
                        if jt == 2:
                            qproj(2)
                        if jt == 6:
                            qproj(3)
                        if jt % 2 == 0 and jt < NJT - 2:
                            vvproj(jt // 2 + 1)
                    if jt == 0:
                        nc.vector.tensor_copy(lacc[ca][:], p_a[:])
                        nc.gpsimd.tensor_copy(lacc[cb][:], p_b[:])
                    elif jt < NJT - 2:
                        nc.vector.tensor_add(lacc[ca][:], lacc[ca][:], p_a[:])
                        nc.gpsimd.tensor_add(lacc[cb][:], lacc[cb][:], p_b[:])
                    if sc == 1 and jt == 6:
                        finalize0(0)
                    if sc == 1 and jt == 20:
                        make_rx(2)
                        make_rx(3)
                if sc == 0:
                    make_rx(0)
                    make_rx(1)
                    # evacuate pv PSUM so super1 can reuse the banks
                    for ci in range(2):
                        for m in range(2):
                            o = resp.tile([128, IC], f32r, tag=f"osb{ci}_{m}",
                                          name=f"osb{ci}_{m}", bufs=1)
                            nc.vector.tensor_copy(o[:], pv_ps[ci][m][:])
                            osb[2 * sc + ci][m] = o
                else:
                    for r in range(16):
                        w_ps = ps.tile([128, IC], f32, tag="s", name="warm",
                                       bufs=4)
                        nc.tensor.matmul(w_ps[:], ones16, jk[:], start=True,
                                         stop=True)
                    # closing finalize: Vector does the PSUM-reading muls
                    # (GpSimd cannot access PSUM) + m=0 adds; GpSimd does the
                    # m=1 adds; bf16 res halves the exposed out-DMA bytes and
                    # each m's merged [128,1024] DMA fires right after its
                    # own adds
                    scl = [[resp.tile([128, IC], f32, tag=f"scl{ci}_{m}",
                                      name=f"scl{ci}_{m}", bufs=1)
                            for m in range(2)] for ci in range(2)]
                    for ci in range(2):
                        for m in range(2):
                            nc.vector.tensor_mul(scl[ci][m][:],
                                                 pv_ps[ci][m][:],
                                                 rbt[2 * sc + ci][:])
                    res1 = [resp.tile([128, 1024], bf16, tag=f"res1_{m}",
                                      name=f"res1_{m}", bufs=1)
                            for m in range(2)]
                    for ci in range(2):
                        nc.vector.tensor_add(
                            res1[0][:, ci * IC:(ci + 1) * IC],
                            scl[ci][0][:], rx[2 * sc + ci][0][:])
                    nc.sync.dma_start(out_e[0, :, 2 * sc * IC:
                                            (2 * sc + 2) * IC], res1[0][:])
                    for ci in range(2):
                        nc.gpsimd.tensor_add(
                            res1[1][:, ci * IC:(ci + 1) * IC],
                            scl[ci][1][:], rx[2 * sc + ci][1][:])
                    nc.sync.dma_start(out_e[1, :, 2 * sc * IC:
                                            (2 * sc + 2) * IC], res1[1][:])

    nc.compile()
    return nc


def _prep_inputs(x, gn_gamma, gn_beta, wq, bq, wk, bk, wv, bv, wo, bo):
    f = np.float32
    constR = np.zeros((128, _RCOLS), np.float16)
    wov = (wo.astype(f) @ wv.astype(f))
    for base, w in ((_RQ, wq), (_RK, wk), (_RO, wov)):
        wT = w.astype(f).T  # [c_in, c_out]
        for t in range(2):
            constR[:, base + 256 * t:base + 256 * t + 256] = \
                wT[128 * t:128 * (t + 1), :]
    constF = np.zeros((128, _FCOLS), f)
    gsel = np.zeros((2, 128, GROUPS), f)
    gselT = np.zeros((2, GROUPS, 128), f)
    for t in range(2):
        for p in range(128):
            g = (t * 128 + p) // CPG
            gsel[t, p, g] = 1.0
            gselT[t, g, p] = 1.0
    for t in range(2):
        constF[:, _FGS + 32 * t:_FGS + 32 * t + 32] = gsel[t]
        constF[0:GROUPS, _FGT + 128 * t:_FGT + 128 * t + 128] = gselT[t]
    wobvbo = (wo.astype(f) @ bv.astype(f) + bo.astype(f))
    vecs = (gn_gamma, gn_beta, bq, wobvbo)
    for i, v in enumerate(vecs):
        vv = v.astype(f).reshape(2, 128)
        for t in range(2):
            constF[:, _FVEC + 2 * i + t] = vv[t]
    constF[:, _FKC + 0] = -SHIFT
    constF[:, _FKC + 1] = EPS

    common = dict(constR=constR, constF=constF)
    xb = x.reshape(B, C, N).astype(np.float16)
    in_maps = []
    for core in range(NCORES):
        bi, qh = core // 2, core % 2
        xcore = xb[bi]
        if qh:
            xcore = np.concatenate([xcore[:, NQ:], xcore[:, :NQ]], axis=1)
        in_maps.append(dict(x=np.ascontiguousarray(xcore.reshape(2, 128, N)),
                            **common))
    return in_maps


def _execute(inputs, trace=False, **kw):
    from concourse.bass_utils import run_bass_kernel_spmd
    if "nc" not in _cache:
        _cache["nc"] = _build()
    nc = _cache["nc"]
    in_maps = _prep_inputs(**inputs)
    res = run_bass_kernel_spmd(nc, in_maps, core_ids=list(range(NCORES)),
                               trace=trace, **kw)
    out = np.empty((B, C, N), np.float32)
    for core in range(NCORES):
        bi, qh = core // 2, core % 2
        chunk = np.asarray(res.results[core]["out"]).astype(
            np.float32).reshape(C, NQ)
        out[bi, :, qh * NQ:(qh + 1) * NQ] = chunk
    return out.reshape(B, C, DD, HH, WW), res


def kernel(**inputs):
    out, _ = _execute(inputs, trace=False)
    return out
